# revision 1
# baseline (speedup 1.0000x reference)
import sys
sys.path.insert(0, '/opt/trn_rl_repo')
import numpy as np
import concourse.bass as bass
import concourse.bacc as bacc
import concourse.tile as tile
from concourse import mybir
from concourse.bass_utils import run_bass_kernel_spmd

F32 = mybir.dt.float32
AF = mybir.ActivationFunctionType
ALU = mybir.AluOpType

U = 400        # LSTM units
KATT = 10     # attention gaussians
NCHARS = 73   # alphabet
NMIX = 20     # GMM components
UC = 50       # char positions
NB = 4        # batch per core
NCORES = 8
NOUT = 6 * NMIX + 1  # 121

# m-tiles: per gate [128,128,128,16] -> 16 m-tiles, psum z [128, 64]
MW = [128, 128, 128, 16] * 4
MOFF = [512 * (m // 4) + 128 * (m % 4) for m in range(16)]   # psum-layout offsets
SOFF = [400 * (m // 4) + 128 * (m % 4) for m in range(16)]   # storage offsets (unpadded)
XWROWS = 99  # xw moving tile: w @0:73, junk, x @96:99

_CACHE = {}


def _pack_wblocks(W, nkt):
    """W [rows<=128*nkt, 1600 cols] -> [128, nkt*1600] k-major."""
    out = np.zeros((128, nkt * 1600), np.float32)
    r = W.shape[0]
    for k in range(nkt):
        lo, hi = 128 * k, min(128 * (k + 1), r)
        if lo >= r:
            break
        out[0 : hi - lo, k * 1600 : k * 1600 + 1600] = W[lo:hi]
    return out


def _build_program(T):
    nc = bacc.Bacc("TRN2", target_bir_lowering=False, debug=False, num_devices=NCORES)

    dW1 = nc.dram_tensor("W1", [128, 5 * 1600], F32, kind="ExternalInput").ap()
    dW2 = nc.dram_tensor("W2", [128, 9 * 1600], F32, kind="ExternalInput").ap()
    dW3 = nc.dram_tensor("W3", [128, 9 * 1600], F32, kind="ExternalInput").ap()
    dPB = nc.dram_tensor("PB", [128, 9 * 16], F32, kind="ExternalInput").ap()
    dXT = nc.dram_tensor("XT", [3, T * NB], F32, kind="ExternalInput").ap()
    dWATT = nc.dram_tensor("WATT", [128, 4 * 30], F32, kind="ExternalInput").ap()
    dV3 = nc.dram_tensor("V3", [1, 3 * UC], F32, kind="ExternalInput").ap()
    dOHB = nc.dram_tensor("OHB", [UC, NB * NCHARS], F32, kind="ExternalInput").ap()
    dWMDN = nc.dram_tensor("WMDN", [128, 4 * 200], F32, kind="ExternalInput").ap()
    dHB = nc.dram_tensor("HB", [128, 16], F32, kind="ExternalInput").ap()
    dOUT1 = nc.dram_tensor("OUT1", [128, T * NB], F32, kind="ExternalOutput").ap()
    dOUT2 = nc.dram_tensor("OUT2", [72, T * NB], F32, kind="ExternalOutput").ap()
    import os as _os
    _DBG = bool(_os.environ.get("KDBG"))
    if _DBG:
        dDH3 = nc.dram_tensor("DH3", [128, T * 16], F32, kind="ExternalOutput").ap()
        dDH1 = nc.dram_tensor("DH1", [128, 16], F32, kind="ExternalOutput").ap()
        dDC1 = nc.dram_tensor("DC1", [128, 16], F32, kind="ExternalOutput").ap()
        dDKAP = nc.dram_tensor("DKAP", [1, 40], F32, kind="ExternalOutput").ap()
        dDXW = nc.dram_tensor("DXW", [XWROWS, NB], F32, kind="ExternalOutput").ap()

    with tile.TileContext(nc) as tc:
        with tc.tile_pool(name="statics", bufs=1) as statics, \
             tc.tile_pool(name="states", bufs=1) as states:

            sW1 = statics.tile([128, 5 * 1600], F32)
            sW2 = statics.tile([128, 9 * 1600], F32)
            sW3 = statics.tile([128, 9 * 1600], F32)
            sPB = statics.tile([128, 9 * 16], F32)
            sXT = statics.tile([3, T * NB], F32)
            sWATT = statics.tile([128, 4 * 30], F32)
            sV3 = statics.tile([1, 3 * UC], F32)
            sOHB = statics.tile([UC, NB * NCHARS], F32)
            sWMDN = statics.tile([128, 4 * 200], F32)
            sHB = statics.tile([128, 16], F32)
            for dst, src in [(sW1, dW1), (sW2, dW2), (sW3, dW3), (sPB, dPB),
                             (sXT, dXT), (sWATT, dWATT), (sV3, dV3),
                             (sOHB, dOHB), (sWMDN, dWMDN), (sHB, dHB)]:
                nc.gpsimd.dma_start(out=dst[:], in_=src[:])

            h3all = states.tile([128, T * 16], F32)
            OUTS1 = states.tile([128, T * NB], F32)
            OUTS2 = states.tile([72, T * NB], F32)
            h = [states.tile([128, 16], F32, name=f"h{i}") for i in range(3)]
            c = [states.tile([128, 16], F32, name=f"c{i}") for i in range(3)]
            xw = [states.tile([XWROWS, NB], F32, name=f"xw{i}") for i in range(3)]
            kap = states.tile([1, 4 * KATT], F32)  # (b, k)
            for tl in c + xw:
                nc.vector.memset(tl[:], 0.0)
            nc.vector.memset(kap[:], 0.0)
            nc.vector.memset(h3all[:], 0.0)
            # h init: zeros except bias constant 1.0 at (p96, blk3 cols)
            for tl in h:
                nc.vector.tensor_copy(out=tl[:], in_=sHB[:])
            nc.vector.memset(h3all[96:128, :], 1.0)

            with tc.tile_pool(name="psum", bufs=1, space="PSUM") as psum, \
                 tc.tile_pool(name="scratch", bufs=2) as scratch:

                z = [psum.tile([128, 64], F32, name=f"z{i}") for i in range(3)]
                attp = psum.tile([1, NB * 30], F32)
                argp = psum.tile([UC, NB * KATT], F32)
                wp = psum.tile([NCHARS, NB], F32)
                for tl in z:
                    nc.vector.memset(tl[:], 0.0)

                _sc = {}
                for _t in ["pc0", "zi_", "ti", "ig", "pc1", "zf_", "tf", "fg",
                           "tg", "m1", "m2", "pc2", "zo_", "to", "og", "tcn"]:
                    _sc[_t] = scratch.tile([128, 16], F32, tag=_t, name=_t)
                for _t, _shp in [("E", [1, NB * 30]), ("kap2", [1, NB * KATT]),
                                 ("bk", [1, NB * KATT]), ("A_", [1, NB * KATT]),
                                 ("bk2", [1, NB * KATT]), ("B_", [1, NB * KATT]),
                                 ("C_", [1, NB * KATT]), ("P", [UC, NB * KATT]),
                                 ("phi", [UC, NB])]:
                    _sc[_t] = scratch.tile(_shp, F32, tag=_t, name=_t)

                def layer_mms(l, sW, movs):
                    for m in range(16):
                        w0, mw = int(SOFF[m]), MW[m]
                        for ki, (mov, kidx) in enumerate(movs):
                            kp = mov.shape[0]
                            nc.tensor.matmul(
                                z[l][0:mw, 4 * m : 4 * m + 4],
                                sW[0:kp, kidx * 1600 + w0 : kidx * 1600 + w0 + mw],
                                mov,
                                start=(ki == 0),
                                stop=(ki == len(movs) - 1),
                            )

                def st(tag):
                    return _sc[tag]

                def cell(l, t):
                    zt = z[l]
                    zi, zf = zt[:, 0:16], zt[:, 16:32]
                    zg, zo = zt[:, 32:48], zt[:, 48:64]
                    p0 = sPB[:, (3 * l + 0) * 16 : (3 * l + 0) * 16 + 16]
                    p1 = sPB[:, (3 * l + 1) * 16 : (3 * l + 1) * 16 + 16]
                    p2 = sPB[:, (3 * l + 2) * 16 : (3 * l + 2) * 16 + 16]
                    ct = c[l]
                    pc0 = st("pc0")
                    nc.vector.tensor_tensor(out=pc0[:], in0=p0, in1=ct[:], op=ALU.mult)
                    zi_ = st("zi_")
                    nc.vector.tensor_tensor(out=zi_[:], in0=zi, in1=pc0[:], op=ALU.add)
                    ti = st("ti")
                    nc.scalar.activation(out=ti[:], in_=zi_[:], func=AF.Tanh, scale=0.5)
                    ig = st("ig")
                    nc.vector.tensor_scalar(out=ig[:], in0=ti[:], scalar1=0.5,
                                            scalar2=0.5, op0=ALU.mult, op1=ALU.add)
                    pc1 = st("pc1")
                    nc.vector.tensor_tensor(out=pc1[:], in0=p1, in1=ct[:], op=ALU.mult)
                    zf_ = st("zf_")
                    nc.vector.tensor_tensor(out=zf_[:], in0=zf, in1=pc1[:], op=ALU.add)
                    tf = st("tf")
                    nc.scalar.activation(out=tf[:], in_=zf_[:], func=AF.Tanh, scale=0.5)
                    fg = st("fg")
                    nc.vector.tensor_scalar(out=fg[:], in0=tf[:], scalar1=0.5,
                                            scalar2=0.5, op0=ALU.mult, op1=ALU.add)
                    tg = st("tg")
                    nc.scalar.activation(out=tg[:], in_=zg, func=AF.Tanh)
                    m1 = st("m1")
                    nc.vector.tensor_tensor(out=m1[:], in0=ig[:], in1=tg[:], op=ALU.mult)
                    m2 = st("m2")
                    nc.vector.tensor_tensor(out=m2[:], in0=fg[:], in1=ct[:], op=ALU.mult)
                    nc.vector.tensor_tensor(out=ct[:], in0=m1[:], in1=m2[:], op=ALU.add)
                    pc2 = st("pc2")
                    nc.vector.tensor_tensor(out=pc2[:], in0=p2, in1=ct[:], op=ALU.mult)
                    zo_ = st("zo_")
                    nc.vector.tensor_tensor(out=zo_[:], in0=zo, in1=pc2[:], op=ALU.add)
                    to = st("to")
                    nc.scalar.activation(out=to[:], in_=zo_[:], func=AF.Tanh, scale=0.5)
                    og = st("og")
                    nc.vector.tensor_scalar(out=og[:], in0=to[:], scalar1=0.5,
                                            scalar2=0.5, op0=ALU.mult, op1=ALU.add)
                    tcn = st("tcn")
                    nc.scalar.activation(out=tcn[:], in_=ct[:], func=AF.Tanh)
                    # h update: blocks 0-2 full; blk3 only partitions 0:96
                    nc.vector.tensor_tensor(out=h[l][:, 0:12], in0=og[:, 0:12],
                                            in1=tcn[:, 0:12], op=ALU.mult)
                    nc.vector.tensor_tensor(out=h[l][0:96, 12:16], in0=og[0:96, 12:16],
                                            in1=tcn[0:96, 12:16], op=ALU.mult)
                    if l == 2:
                        nc.vector.tensor_copy(
                            out=h3all[:, bass.DynSlice(t * 16, 12)],
                            in_=h[2][:, 0:12])
                        nc.vector.tensor_copy(
                            out=h3all[0:96, bass.DynSlice(t * 16 + 12, 4)],
                            in_=h[2][0:96, 12:16])

                def attention():
                    for b in range(NB):
                        for k in range(4):
                            nc.tensor.matmul(
                                attp[0:1, 30 * b : 30 * b + 30],
                                h[0][:, 4 * k + b : 4 * k + b + 1],
                                sWATT[:, 30 * k : 30 * k + 30],
                                start=(k == 0), stop=(k == 3),
                            )
                    E = _sc["E"]
                    nc.scalar.activation(out=E[:], in_=attp[:], func=AF.Exp)
                    Ev = E[0:1, :].rearrange("p (b x) -> p b x", b=NB)
                    av = attp[0:1, :].rearrange("p (b x) -> p b x", b=NB)
                    kapv = kap[0:1, :].rearrange("p (b k) -> p b k", b=NB)
                    kap2 = _sc["kap2"]
                    k2v = kap2[0:1, :].rearrange("p (b k) -> p b k", b=NB)
                    nc.vector.tensor_tensor(out=k2v, in0=kapv, in1=Ev[:, :, 20:30],
                                            op=ALU.add)
                    nc.vector.tensor_copy(out=kap[:], in_=kap2[:])
                    bk = _sc["bk"]
                    bkv = bk[0:1, :].rearrange("p (b k) -> p b k", b=NB)
                    nc.vector.tensor_tensor(out=bkv, in0=Ev[:, :, 10:20], in1=k2v,
                                            op=ALU.mult)
                    A_ = _sc["A_"]
                    Av = A_[0:1, :].rearrange("p (b k) -> p b k", b=NB)
                    bk2 = _sc["bk2"]
                    bk2v = bk2[0:1, :].rearrange("p (b k) -> p b k", b=NB)
                    nc.vector.tensor_tensor(out=bk2v, in0=bkv, in1=k2v, op=ALU.mult)
                    nc.vector.tensor_tensor(out=Av, in0=av[:, :, 0:10], in1=bk2v,
                                            op=ALU.subtract)
                    B_ = _sc["B_"]
                    nc.vector.tensor_scalar(out=B_[:], in0=bk[:], scalar1=2.0,
                                            scalar2=None, op0=ALU.mult)
                    C_ = _sc["C_"]
                    Cv = C_[0:1, :].rearrange("p (b k) -> p b k", b=NB)
                    nc.vector.tensor_scalar(out=Cv, in0=Ev[:, :, 10:20], scalar1=-1.0,
                                            scalar2=None, op0=ALU.mult)
                    # arg[u,(b,k)] = A + u*B + u^2*C  via 3 accumulating K=1 matmuls
                    nc.tensor.matmul(argp[:], sV3[0:1, 0:UC], A_[:],
                                     start=True, stop=False)
                    nc.tensor.matmul(argp[:], sV3[0:1, UC:2 * UC], B_[:],
                                     start=False, stop=False)
                    nc.tensor.matmul(argp[:], sV3[0:1, 2 * UC:3 * UC], C_[:],
                                     start=False, stop=True)
                    P = _sc["P"]
                    nc.scalar.activation(out=P[:], in_=argp[:], func=AF.Exp)
                    phi = _sc["phi"]
                    Pv = P[:, :].rearrange("p (b k) -> p b k", b=NB)
                    nc.vector.tensor_reduce(out=phi[:], in_=Pv,
                                            axis=mybir.AxisListType.X, op=ALU.add)
                    for b in range(NB):
                        nc.tensor.matmul(
                            wp[:, b : b + 1],
                            sOHB[:, NCHARS * b : NCHARS * b + NCHARS],
                            phi[:, b : b + 1],
                            start=True, stop=True,
                        )
                    for l in range(3):
                        nc.vector.tensor_copy(out=xw[l][0:73, :], in_=wp[:])

                xstage = scratch.tile([3, NB], F32, tag="xstage", name="xstage")
                import os as _os
                import contextlib as _cl
                _pyloop = bool(_os.environ.get("KPYLOOP"))

                def _loop():
                    if _pyloop:
                        return _cl.nullcontext(range(T))
                    return tc.For_i(0, T)

                with _loop() as _ts:
                    _titer = _ts if _pyloop else [_ts]
                    for t in _titer:
                        nc.vector.tensor_copy(out=xstage[:], in_=sXT[0:3, bass.ts(t, NB)])
                        for l in range(3):
                            nc.vector.tensor_copy(out=xw[l][96:99, :], in_=xstage[0:3, :])
                        layer_mms(0, sW1, [(h[0][:, 0:4], 0), (h[0][:, 4:8], 1),
                                           (h[0][:, 8:12], 2), (h[0][:, 12:16], 3),
                                           (xw[0][:], 4)])
                        cell(0, t)
                        attention()
                        layer_mms(1, sW2, [(h[1][:, 0:4], 0), (h[1][:, 4:8], 1),
                                           (h[1][:, 8:12], 2), (h[1][:, 12:16], 3),
                                           (h[0][:, 0:4], 4), (h[0][:, 4:8], 5),
                                           (h[0][:, 8:12], 6), (h[0][:, 12:16], 7),
                                           (xw[1][:], 8)])
                        cell(1, t)
                        layer_mms(2, sW3, [(h[2][:, 0:4], 0), (h[2][:, 4:8], 1),
                                           (h[2][:, 8:12], 2), (h[2][:, 12:16], 3),
                                           (h[1][:, 0:4], 4), (h[1][:, 4:8], 5),
                                           (h[1][:, 8:12], 6), (h[1][:, 12:16], 7),
                                           (xw[2][:], 8)])
                        cell(2, t)

            # ---- MDN head ----
            # Y1 rows: mu @0:40, eos @64, rho @96:116 ; Y2 rows: pi @0:20, s @32:72
            with tc.tile_pool(name="mpsum", bufs=2, space="PSUM") as mpsum, \
                 tc.tile_pool(name="mscr", bufs=2) as mscr, \
                 tc.tile_pool(name="mones", bufs=1) as mones:
                ones20 = mones.tile([NMIX, 1], F32)
                nc.vector.memset(ones20[:], 1.0)
                ones1_20 = mones.tile([1, NMIX], F32)
                nc.vector.memset(ones1_20[:], 1.0)
                h3v = h3all[:, :].rearrange("p (t x) -> p t x", t=T)
                CC = min(400, T * NB)
                TC = CC // NB
                for ch in range((T + TC - 1) // TC):
                    t0 = TC * ch
                    tn = min(TC, T - t0)
                    cc = tn * NB
                    yp1 = mpsum.tile([128, CC], F32, tag="yp1")
                    yp2 = mpsum.tile([72, CC], F32, tag="yp2")
                    for k in range(4):
                        nc.tensor.matmul(
                            yp1[0:128, 0:cc],
                            sWMDN[:, 200 * k : 200 * k + 128],
                            h3v[:, t0 : t0 + tn, 4 * k : 4 * k + 4],
                            start=(k == 0), stop=(k == 3))
                    for k in range(4):
                        nc.tensor.matmul(
                            yp2[0:72, 0:cc],
                            sWMDN[:, 200 * k + 128 : 200 * k + 200],
                            h3v[:, t0 : t0 + tn, 4 * k : 4 * k + 4],
                            start=(k == 0), stop=(k == 3))
                    o1 = OUTS1[:, NB * t0 : NB * t0 + cc]
                    o2 = OUTS2[:, NB * t0 : NB * t0 + cc]
                    # pi softmax (pi lives at yp2[0:20])
                    epi = mscr.tile([NMIX, CC], F32, tag="epi")
                    nc.scalar.activation(out=epi[0:NMIX, 0:cc], in_=yp2[0:NMIX, 0:cc],
                                         func=AF.Exp)
                    sp = mpsum.tile([1, CC], F32, tag="sp")
                    nc.tensor.matmul(sp[0:1, 0:cc], ones20[:], epi[0:NMIX, 0:cc],
                                     start=True, stop=True)
                    rec = mscr.tile([1, CC], F32, tag="rec")
                    nc.vector.reciprocal(out=rec[0:1, 0:cc], in_=sp[0:1, 0:cc])
                    bp = mpsum.tile([NMIX, CC], F32, tag="bp")
                    nc.tensor.matmul(bp[0:NMIX, 0:cc], ones1_20[:], rec[0:1, 0:cc],
                                     start=True, stop=True)
                    nc.vector.tensor_tensor(out=o2[0:20, :], in0=epi[0:NMIX, 0:cc],
                                            in1=bp[0:NMIX, 0:cc], op=ALU.mult)
                    # mu copy (yp1[0:40])
                    nc.vector.tensor_copy(out=o1[0:40, :], in_=yp1[0:40, 0:cc])
                    # rho tanh (yp1[96:116])
                    nc.scalar.activation(out=o1[96:116, :], in_=yp1[96:116, 0:cc],
                                         func=AF.Tanh)
                    # eos sigmoid via tanh (yp1[64:65])
                    teos = mscr.tile([65, CC], F32, tag="teos")
                    nc.scalar.activation(out=teos[64:65, 0:cc], in_=yp1[64:65, 0:cc],
                                         func=AF.Tanh, scale=0.5)
                    nc.vector.tensor_scalar(out=o1[64:65, :], in0=teos[64:65, 0:cc],
                                            scalar1=0.5, scalar2=0.5,
                                            op0=ALU.mult, op1=ALU.add)
                    # s exp (yp2[32:72], split at quadrant boundary)
                    nc.scalar.activation(out=o2[32:64, :], in_=yp2[32:64, 0:cc],
                                         func=AF.Exp)
                    nc.scalar.activation(out=o2[64:72, :], in_=yp2[64:72, 0:cc],
                                         func=AF.Exp)
            nc.gpsimd.dma_start(out=dOUT1[:], in_=OUTS1[:])
            nc.gpsimd.dma_start(out=dOUT2[:], in_=OUTS2[:])
            if _DBG:
                nc.gpsimd.dma_start(out=dDH3, in_=h3all[:])
                nc.gpsimd.dma_start(out=dDH1, in_=h[0][:])
                nc.gpsimd.dma_start(out=dDC1, in_=c[0][:])
                nc.gpsimd.dma_start(out=dDKAP, in_=kap[:])
                nc.gpsimd.dma_start(out=dDXW, in_=xw[0][:])

    nc.compile()
    return nc


def _prep_core(inputs, bsl, T):
    x = np.asarray(inputs['input_strokes'], np.float32)
    chars = np.asarray(inputs['input_chars'])
    lens = np.asarray(inputs['input_char_lens'])

    def W_of(l):
        if l == 0:
            Wx = np.asarray(inputs['Wx0'], np.float32)
            rows = [np.asarray(inputs['Wh0'], np.float32)]
        else:
            Wx = np.asarray(inputs['Wx%d' % l], np.float32)
            rows = [np.asarray(inputs['Wh%d' % l], np.float32), Wx[76:476]]
        b = np.asarray(inputs['b%d' % l], np.float32)
        nh = len(rows)
        nkt = 4 * nh + 1
        Wfull = np.zeros((128 * nkt, 1600), np.float32)
        for j, Whx in enumerate(rows):
            Wfull[512 * j : 512 * j + 400] = Whx
        Wfull[3 * 128 + 96] = b             # bias via h-self blk3 p96 == 1.0
        base = 512 * nh                     # xw k-tile: w @0:73, x @96:99
        Wfull[base : base + 73] = Wx[3:76]
        Wfull[base + 96 : base + 99] = Wx[0:3]
        return _pack_wblocks(Wfull, nkt)

    W1, W2, W3 = W_of(0), W_of(1), W_of(2)

    PB = np.zeros((128, 9 * 16), np.float32)
    for l in range(3):
        p = np.asarray(inputs['p%d' % l], np.float32)
        for j in range(3):
            pbv = np.zeros((128, 16), np.float32)
            for blk in range(4):
                n = min(128, 400 - 128 * blk)
                pbv[0:n, 4 * blk : 4 * blk + 4] = p[j][128 * blk : 128 * blk + n, None]
            PB[:, (3 * l + j) * 16 : (3 * l + j) * 16 + 16] = pbv

    XT = np.zeros((3, T * NB), np.float32)
    xs = x[bsl]
    for b in range(NB):
        XT[:, b::NB] = xs[b].T
    WATT = np.zeros((128, 4 * 30), np.float32)
    wa = np.asarray(inputs['W_att'], np.float32)
    for k in range(4):
        n = min(128, 400 - 128 * k)
        WATT[0:n, 30 * k : 30 * k + 30] = wa[128 * k : 128 * k + n]
    WATT[96, 90:120] = np.asarray(inputs['b_att'], np.float32)
    V3 = np.concatenate([np.ones(UC), np.arange(UC),
                         np.arange(UC) ** 2]).astype(np.float32)[None, :]
    OHB = np.zeros((UC, NB * NCHARS), np.float32)
    for b, gb in enumerate(bsl):
        oh = np.zeros((UC, NCHARS), np.float32)
        oh[np.arange(UC), chars[gb].astype(int)] = 1.0
        oh[int(lens[gb]):] = 0.0
        OHB[:, NCHARS * b : NCHARS * b + NCHARS] = oh
    # WMDN: per k-tile block [m1(128) | m2(72)]
    wm = np.asarray(inputs['W_mdn'], np.float32)
    bm = np.asarray(inputs['b_mdn'], np.float32)
    wmf = np.zeros((512, 121), np.float32)
    wmf[0:400] = wm
    wmf[3 * 128 + 96] = bm                  # bias via h3all p96 blk3 == 1.0
    m1 = np.zeros((512, 128), np.float32)
    m2 = np.zeros((512, 72), np.float32)
    m1[:, 0:40] = wmf[:, 20:60]             # mu1, mu2
    m1[:, 64:65] = wmf[:, 120:121]          # eos
    m1[:, 96:116] = wmf[:, 100:120]         # rho
    m2[:, 0:20] = wmf[:, 0:20]              # pi
    m2[:, 32:72] = wmf[:, 60:100]           # s1, s2
    WMDN = np.zeros((128, 4 * 200), np.float32)
    for k in range(4):
        WMDN[:, 200 * k : 200 * k + 128] = m1[128 * k : 128 * k + 128]
        WMDN[:, 200 * k + 128 : 200 * k + 200] = m2[128 * k : 128 * k + 128]
    HB = np.zeros((128, 16), np.float32)
    HB[96, 12:16] = 1.0
    return {'W1': W1, 'W2': W2, 'W3': W3, 'PB': PB, 'XT': XT, 'WATT': WATT,
            'V3': V3, 'OHB': OHB, 'WMDN': WMDN, 'HB': HB}


def kernel(**inputs):
    x = np.asarray(inputs['input_strokes'])
    B, T, _ = x.shape
    if T not in _CACHE:
        _CACHE[T] = _build_program(T)
    nc = _CACHE[T]
    in_maps = [_prep_core(inputs, list(range(cr * NB, cr * NB + NB)), T)
               for cr in range(NCORES)]
    res = run_bass_kernel_spmd(nc, in_maps, list(range(NCORES)))
    outs = []
    for cr in range(NCORES):
        O1 = res.results[cr]['OUT1'].reshape(128, T, NB)
        O2 = res.results[cr]['OUT2'].reshape(72, T, NB)
        y = np.empty((NB, T, NOUT), np.float32)
        y[..., 0:20] = O2[0:20].transpose(2, 1, 0)
        y[..., 20:60] = O1[0:40].transpose(2, 1, 0)
        y[..., 60:100] = O2[32:72].transpose(2, 1, 0)
        y[..., 100:120] = O1[96:116].transpose(2, 1, 0)
        y[..., 120:121] = O1[64:65].transpose(2, 1, 0)
        outs.append(y)
    return np.concatenate(outs, 0).astype(np.float32)



# revision 12
# speedup vs baseline: 6.6492x; 6.6492x over previous
import sys
sys.path.insert(0, '/opt/trn_rl_repo')
import numpy as np
import ml_dtypes
import concourse.bass as bass
import concourse.bacc as bacc
import concourse.tile as tile
from concourse import mybir
from concourse.bass_utils import run_bass_kernel_spmd

F32 = mybir.dt.float32
BF16 = mybir.dt.float16
NPBF = np.float16
AF = mybir.ActivationFunctionType
ALU = mybir.AluOpType

U = 400        # LSTM units
KATT = 10     # attention gaussians
NCHARS = 73   # alphabet
NMIX = 20     # GMM components
UC = 50       # char positions
NB = 4        # batch per core
NCORES = 8
NOUT = 6 * NMIX + 1  # 121

# m-tiles: per gate [128,128,128,16] -> 16 m-tiles, psum z [128, 64]
MW = [128, 128, 128, 16] * 4
SOFF = [400 * (m // 4) + 128 * (m % 4) for m in range(16)]   # weight-col offsets

# Moving k-tile layouts (partition-write-alignment legal):
#  cmbA0 [128,4]: w(t-1)@0:73 | b0-one@80 | x(t)@96:99
#  cmbA1 [128,4]: w(t)@0:73 | x(t)@73:76 | h2b3(t-1)@76:92 | b1-one@92 | h1b3(t)@96:112
#  x12   [128,4]: h3b3(t-1)@0:16 | h2b3(t)@32:48 | h1b3(t-1)@64:80 | b2-one@96
#  phi_aug [128,4]: x(t)@0:3 | h2b3(t-1)@32:48 | phi@64:114   (window matmul moving)
#  wpA psum [92,4] = OH_aug^T @ phi_aug = w@0:73 | x@73:76 | h2b3@76:92
OHROWS = 114
OHCOLS = 92

_CACHE = {}


def _build_program(T):
    nc = bacc.Bacc("TRN2", target_bir_lowering=False, debug=False, num_devices=NCORES)

    dW1 = nc.dram_tensor("W1", [128, 5 * 1600], BF16, kind="ExternalInput").ap()
    dW2 = nc.dram_tensor("W2", [128, 7 * 1600], BF16, kind="ExternalInput").ap()
    dW3 = nc.dram_tensor("W3", [128, 8 * 1600], BF16, kind="ExternalInput").ap()
    dPB = nc.dram_tensor("PB", [128, 9 * 16], F32, kind="ExternalInput").ap()
    dXT = nc.dram_tensor("XT", [3, T * NB], BF16, kind="ExternalInput").ap()
    dWATT = nc.dram_tensor("WATT", [128, 4 * 30], BF16, kind="ExternalInput").ap()
    dV3 = nc.dram_tensor("V3", [1, 3 * UC], F32, kind="ExternalInput").ap()
    dOHB = nc.dram_tensor("OHB", [OHROWS, NB * OHCOLS], BF16, kind="ExternalInput").ap()
    dWMDN = nc.dram_tensor("WMDN", [128, 4 * 200], BF16, kind="ExternalInput").ap()
    dHB = nc.dram_tensor("HB", [128, 16], BF16, kind="ExternalInput").ap()
    dCI = nc.dram_tensor("CI", [128, 12], BF16, kind="ExternalInput").ap()
    dOUT1 = nc.dram_tensor("OUT1", [128, T * NB], F32, kind="ExternalOutput").ap()
    dOUT2 = nc.dram_tensor("OUT2", [72, T * NB], F32, kind="ExternalOutput").ap()

    with tile.TileContext(nc) as tc:
        with tc.tile_pool(name="statics", bufs=1) as statics, \
             tc.tile_pool(name="states", bufs=1) as states:

            sW1 = statics.tile([128, 5 * 1600], BF16)
            sW2 = statics.tile([128, 7 * 1600], BF16)
            sW3 = statics.tile([128, 8 * 1600], BF16)
            sPB = statics.tile([128, 9 * 16], F32)
            sXT = statics.tile([3, T * NB], BF16)
            sWATT = statics.tile([128, 4 * 30], BF16)
            sV3 = statics.tile([1, 3 * UC], F32)
            sOHB = statics.tile([OHROWS, NB * OHCOLS], BF16)
            sWMDN = statics.tile([128, 4 * 200], BF16)
            sHB = statics.tile([128, 16], BF16)
            for dst, src in [(sW1, dW1), (sW2, dW2), (sW3, dW3), (sPB, dPB),
                             (sXT, dXT), (sWATT, dWATT), (sV3, dV3),
                             (sOHB, dOHB), (sWMDN, dWMDN), (sHB, dHB)]:
                nc.gpsimd.dma_start(out=dst[:], in_=src[:])

            h3all = states.tile([128, T * 16], BF16)
            OUTS1 = states.tile([128, T * NB], F32)
            OUTS2 = states.tile([72, T * NB], F32)
            h = [states.tile([128, 16], BF16, name=f"h{i}") for i in range(3)]
            c = [states.tile([128, 16], F32, name=f"c{i}") for i in range(3)]
            cmbA0 = states.tile([128, NB], BF16, name="cmbA0")
            cmbA1 = states.tile([128, NB], BF16, name="cmbA1")
            x12 = states.tile([128, NB], BF16, name="x12")
            phi_aug = states.tile([128, NB], BF16, name="phi_aug")
            # DMA-init combo tiles: zeros + bias-one rows (avoids unaligned writes)
            nc.gpsimd.dma_start(out=cmbA0[:], in_=dCI[:, 0:4])
            nc.gpsimd.dma_start(out=cmbA1[:], in_=dCI[:, 4:8])
            nc.gpsimd.dma_start(out=x12[:], in_=dCI[:, 8:12])
            kap = states.tile([1, 4 * KATT], F32)  # (b, k)
            for tl in c:
                nc.vector.memset(tl[:], 0.0)
            nc.vector.memset(phi_aug[:], 0.0)
            nc.vector.memset(kap[:], 0.0)
            nc.vector.memset(h3all[:], 0.0)
            # h init: zeros except bias constant 1.0 at (p96, blk3 cols)
            for tl in h:
                nc.vector.tensor_copy(out=tl[:], in_=sHB[:])
            nc.vector.memset(h3all[96:128, :], 1.0)

            with tc.tile_pool(name="psum", bufs=1, space="PSUM") as psum, \
                 tc.tile_pool(name="scratch", bufs=2) as scratch:

                z = [psum.tile([128, 64], F32, name=f"z{i}") for i in range(3)]
                attp = psum.tile([1, NB * 30], F32)
                argp = psum.tile([UC, NB * KATT], F32)
                wpA = psum.tile([OHCOLS, NB], F32)
                for tl in z:
                    nc.vector.memset(tl[:], 0.0)

                _sc = {}
                for _t in ["pc0", "zi_", "ti", "ig", "pc1", "zf_", "tf", "fg",
                           "tg", "m1", "m2", "pc2", "zo_", "to", "og", "tcn",
                           "hb3"]:
                    _sc[_t] = scratch.tile([128, 16], F32, tag=_t, name=_t)
                for _t, _shp, _dt in [("E", [1, NB * 30], F32),
                                      ("kap2", [1, NB * KATT], F32),
                                      ("bk", [1, NB * KATT], F32),
                                      ("A_", [1, NB * KATT], F32),
                                      ("bk2", [1, NB * KATT], F32),
                                      ("B_", [1, NB * KATT], F32),
                                      ("C_", [1, NB * KATT], F32),
                                      ("P", [UC, NB * KATT], BF16)]:
                    _sc[_t] = scratch.tile(_shp, _dt, tag=_t, name=_t)

                def layer_mms(l, sW, movs, first, last):
                    # start=True clears has_written for the WHOLE psum bank,
                    # so a step may carry exactly ONE start per z tile: the
                    # very first matmul (bank clear -> every region's first
                    # touch overwrites, later ones accumulate). This lets the
                    # recurrence-independent partials issue early without
                    # corrupting open accumulations in the same bank.
                    for m in range(16):
                        w0, mw = int(SOFF[m]), MW[m]
                        for ki, (mov, kidx) in enumerate(movs):
                            kp = mov.shape[0]
                            nc.tensor.matmul(
                                z[l][0:mw, 4 * m : 4 * m + 4],
                                sW[0:kp, kidx * 1600 + w0 : kidx * 1600 + w0 + mw],
                                mov,
                                start=(first and m == 0 and ki == 0),
                                stop=(last and ki == len(movs) - 1),
                                skip_group_check=True,
                            )

                def st(tag):
                    return _sc[tag]

                def cell(l, t):
                    zt = z[l]
                    zi, zf = zt[:, 0:16], zt[:, 16:32]
                    zg, zo = zt[:, 32:48], zt[:, 48:64]
                    p0 = sPB[:, (3 * l + 0) * 16 : (3 * l + 0) * 16 + 16]
                    p1 = sPB[:, (3 * l + 1) * 16 : (3 * l + 1) * 16 + 16]
                    p2 = sPB[:, (3 * l + 2) * 16 : (3 * l + 2) * 16 + 16]
                    ct = c[l]
                    pc0 = st("pc0")
                    nc.vector.tensor_tensor(out=pc0[:], in0=p0, in1=ct[:], op=ALU.mult)
                    zi_ = st("zi_")
                    nc.vector.tensor_tensor(out=zi_[:], in0=zi, in1=pc0[:], op=ALU.add)
                    ti = st("ti")
                    nc.scalar.activation(out=ti[:], in_=zi_[:], func=AF.Tanh, scale=0.5)
                    ig = st("ig")
                    nc.vector.tensor_scalar(out=ig[:], in0=ti[:], scalar1=0.5,
                                            scalar2=0.5, op0=ALU.mult, op1=ALU.add)
                    pc1 = st("pc1")
                    nc.vector.tensor_tensor(out=pc1[:], in0=p1, in1=ct[:], op=ALU.mult)
                    zf_ = st("zf_")
                    nc.vector.tensor_tensor(out=zf_[:], in0=zf, in1=pc1[:], op=ALU.add)
                    tf = st("tf")
                    nc.scalar.activation(out=tf[:], in_=zf_[:], func=AF.Tanh, scale=0.5)
                    fg = st("fg")
                    nc.vector.tensor_scalar(out=fg[:], in0=tf[:], scalar1=0.5,
                                            scalar2=0.5, op0=ALU.mult, op1=ALU.add)
                    tg = st("tg")
                    nc.scalar.activation(out=tg[:], in_=zg, func=AF.Tanh)
                    m1 = st("m1")
                    nc.vector.tensor_tensor(out=m1[:], in0=ig[:], in1=tg[:], op=ALU.mult)
                    m2 = st("m2")
                    nc.vector.tensor_tensor(out=m2[:], in0=fg[:], in1=ct[:], op=ALU.mult)
                    nc.vector.tensor_tensor(out=ct[:], in0=m1[:], in1=m2[:], op=ALU.add)
                    pc2 = st("pc2")
                    nc.vector.tensor_tensor(out=pc2[:], in0=p2, in1=ct[:], op=ALU.mult)
                    zo_ = st("zo_")
                    nc.vector.tensor_tensor(out=zo_[:], in0=zo, in1=pc2[:], op=ALU.add)
                    to = st("to")
                    nc.scalar.activation(out=to[:], in_=zo_[:], func=AF.Tanh, scale=0.5)
                    og = st("og")
                    nc.vector.tensor_scalar(out=og[:], in0=to[:], scalar1=0.5,
                                            scalar2=0.5, op0=ALU.mult, op1=ALU.add)
                    tcn = st("tcn")
                    nc.scalar.activation(out=tcn[:], in_=ct[:], func=AF.Tanh)
                    # h update
                    nc.vector.tensor_tensor(out=h[l][:, 0:12], in0=og[:, 0:12],
                                            in1=tcn[:, 0:12], op=ALU.mult)
                    if l != 1:
                        nc.vector.tensor_tensor(out=h[l][0:96, 12:16],
                                                in0=og[0:96, 12:16],
                                                in1=tcn[0:96, 12:16], op=ALU.mult)
                    # blk3 (units 384:400) products into combo tiles
                    hb3 = st("hb3")
                    nc.vector.tensor_tensor(out=hb3[0:16, 0:4], in0=og[0:16, 12:16],
                                            in1=tcn[0:16, 12:16], op=ALU.mult)
                    if l == 0:
                        nc.vector.tensor_copy(out=cmbA1[96:112, :], in_=hb3[0:16, 0:4])
                        nc.vector.tensor_copy(out=x12[64:80, :], in_=hb3[0:16, 0:4])
                    elif l == 1:
                        nc.vector.tensor_copy(out=phi_aug[32:48, :], in_=hb3[0:16, 0:4])
                        nc.vector.tensor_copy(out=x12[32:48, :], in_=hb3[0:16, 0:4])
                    else:
                        nc.vector.tensor_copy(out=x12[0:16, :], in_=hb3[0:16, 0:4])
                        nc.vector.tensor_copy(
                            out=h3all[:, bass.DynSlice(t * 16, 12)],
                            in_=h[2][:, 0:12])
                        nc.vector.tensor_copy(
                            out=h3all[0:96, bass.DynSlice(t * 16 + 12, 4)],
                            in_=h[2][0:96, 12:16])

                def attention():
                    for b in range(NB):
                        for k in range(4):
                            nc.tensor.matmul(
                                attp[0:1, 30 * b : 30 * b + 30],
                                h[0][:, 4 * k + b : 4 * k + b + 1],
                                sWATT[:, 30 * k : 30 * k + 30],
                                start=(k == 0), stop=(k == 3),
                            )
                    E = _sc["E"]
                    nc.scalar.activation(out=E[:], in_=attp[:], func=AF.Exp)
                    Ev = E[0:1, :].rearrange("p (b x) -> p b x", b=NB)
                    av = attp[0:1, :].rearrange("p (b x) -> p b x", b=NB)
                    kapv = kap[0:1, :].rearrange("p (b k) -> p b k", b=NB)
                    kap2 = _sc["kap2"]
                    k2v = kap2[0:1, :].rearrange("p (b k) -> p b k", b=NB)
                    nc.vector.tensor_tensor(out=k2v, in0=kapv, in1=Ev[:, :, 20:30],
                                            op=ALU.add)
                    nc.vector.tensor_copy(out=kap[:], in_=kap2[:])
                    bk = _sc["bk"]
                    bkv = bk[0:1, :].rearrange("p (b k) -> p b k", b=NB)
                    nc.vector.tensor_tensor(out=bkv, in0=Ev[:, :, 10:20], in1=k2v,
                                            op=ALU.mult)
                    A_ = _sc["A_"]
                    Av = A_[0:1, :].rearrange("p (b k) -> p b k", b=NB)
                    bk2 = _sc["bk2"]
                    bk2v = bk2[0:1, :].rearrange("p (b k) -> p b k", b=NB)
                    nc.vector.tensor_tensor(out=bk2v, in0=bkv, in1=k2v, op=ALU.mult)
                    nc.vector.tensor_tensor(out=Av, in0=av[:, :, 0:10], in1=bk2v,
                                            op=ALU.subtract)
                    B_ = _sc["B_"]
                    nc.vector.tensor_scalar(out=B_[:], in0=bk[:], scalar1=2.0,
                                            scalar2=None, op0=ALU.mult)
                    C_ = _sc["C_"]
                    Cv = C_[0:1, :].rearrange("p (b k) -> p b k", b=NB)
                    nc.vector.tensor_scalar(out=Cv, in0=Ev[:, :, 10:20], scalar1=-1.0,
                                            scalar2=None, op0=ALU.mult)
                    # arg[u,(b,k)] = A + u*B + u^2*C  via 3 accumulating K=1 matmuls
                    nc.tensor.matmul(argp[:], sV3[0:1, 0:UC], A_[:],
                                     start=True, stop=False)
                    nc.tensor.matmul(argp[:], sV3[0:1, UC:2 * UC], B_[:],
                                     start=False, stop=False)
                    nc.tensor.matmul(argp[:], sV3[0:1, 2 * UC:3 * UC], C_[:],
                                     start=False, stop=True)
                    P = _sc["P"]
                    nc.scalar.activation(out=P[:], in_=argp[:], func=AF.Exp)
                    Pv = P[:, :].rearrange("p (b k) -> p b k", b=NB)
                    with nc.allow_low_precision(reason="phi: sum of 10 pos bf16"):
                        nc.vector.tensor_reduce(out=phi_aug[64:114, :], in_=Pv,
                                                axis=mybir.AxisListType.X, op=ALU.add)
                    # wpA = OH_aug^T @ phi_aug : [w | x | h2b3] assembled in psum
                    for b in range(NB):
                        nc.tensor.matmul(
                            wpA[:, b : b + 1],
                            sOHB[:, OHCOLS * b : OHCOLS * b + OHCOLS],
                            phi_aug[0:OHROWS, b : b + 1],
                            start=True, stop=True,
                        )
                    nc.vector.tensor_copy(out=cmbA1[0:92, :], in_=wpA[0:92, :])
                    nc.vector.tensor_copy(out=cmbA0[0:73, :], in_=wpA[0:73, :])

                import os as _os
                import contextlib as _cl
                _pyloop = bool(_os.environ.get("KPYLOOP"))

                def _loop():
                    if _pyloop:
                        return _cl.nullcontext(range(T))
                    return tc.For_i(0, T)

                with _loop() as _ts:
                    _titer = _ts if _pyloop else [_ts]
                    for t in _titer:
                        nc.vector.tensor_copy(out=cmbA0[96:99, :],
                                              in_=sXT[0:3, bass.ts(t, NB)])
                        nc.vector.tensor_copy(out=phi_aug[0:3, :],
                                              in_=sXT[0:3, bass.ts(t, NB)])
                        layer_mms(0, sW1, [(h[0][:, 0:4], 0), (h[0][:, 4:8], 1),
                                           (h[0][:, 8:12], 2), (cmbA0[:], 3),
                                           (x12[:], 4)], first=True, last=True)
                        # recurrence-independent partials of z2/z3 issue now so
                        # the PE stays busy under the cell-0 vector chain
                        layer_mms(1, sW2, [(h[1][:, 0:4], 0), (h[1][:, 4:8], 1),
                                           (h[1][:, 8:12], 2)],
                                  first=True, last=False)
                        layer_mms(2, sW3, [(h[2][:, 0:4], 0), (h[2][:, 4:8], 1),
                                           (h[2][:, 8:12], 2)],
                                  first=True, last=False)
                        cell(0, t)
                        attention()
                        layer_mms(1, sW2, [(h[0][:, 0:4], 3), (h[0][:, 4:8], 4),
                                           (h[0][:, 8:12], 5), (cmbA1[:], 6)],
                                  first=False, last=True)
                        cell(1, t)
                        layer_mms(2, sW3, [(h[1][:, 0:4], 3), (h[1][:, 4:8], 4),
                                           (h[1][:, 8:12], 5), (cmbA1[:], 6),
                                           (x12[:], 7)],
                                  first=False, last=True)
                        cell(2, t)

            # ---- MDN head ----
            # Y1 rows: mu @0:40, eos @64, rho @96:116 ; Y2 rows: pi @0:20, s @32:72
            with tc.tile_pool(name="mpsum", bufs=2, space="PSUM") as mpsum, \
                 tc.tile_pool(name="mscr", bufs=2) as mscr, \
                 tc.tile_pool(name="mones", bufs=1) as mones:
                ones20 = mones.tile([NMIX, 1], F32)
                nc.vector.memset(ones20[:], 1.0)
                ones1_20 = mones.tile([1, NMIX], F32)
                nc.vector.memset(ones1_20[:], 1.0)
                h3v = h3all[:, :].rearrange("p (t x) -> p t x", t=T)
                CC = min(400, T * NB)
                TC = CC // NB
                for ch in range((T + TC - 1) // TC):
                    t0 = TC * ch
                    tn = min(TC, T - t0)
                    cc = tn * NB
                    yp1 = mpsum.tile([128, CC], F32, tag="yp1")
                    yp2 = mpsum.tile([72, CC], F32, tag="yp2")
                    for k in range(4):
                        nc.tensor.matmul(
                            yp1[0:128, 0:cc],
                            sWMDN[:, 200 * k : 200 * k + 128],
                            h3v[:, t0 : t0 + tn, 4 * k : 4 * k + 4],
                            start=(k == 0), stop=(k == 3))
                    for k in range(4):
                        nc.tensor.matmul(
                            yp2[0:72, 0:cc],
                            sWMDN[:, 200 * k + 128 : 200 * k + 200],
                            h3v[:, t0 : t0 + tn, 4 * k : 4 * k + 4],
                            start=(k == 0), stop=(k == 3))
                    o1 = OUTS1[:, NB * t0 : NB * t0 + cc]
                    o2 = OUTS2[:, NB * t0 : NB * t0 + cc]
                    # pi softmax (pi lives at yp2[0:20])
                    epi = mscr.tile([NMIX, CC], F32, tag="epi")
                    nc.scalar.activation(out=epi[0:NMIX, 0:cc], in_=yp2[0:NMIX, 0:cc],
                                         func=AF.Exp)
                    sp = mpsum.tile([1, CC], F32, tag="sp")
                    nc.tensor.matmul(sp[0:1, 0:cc], ones20[:], epi[0:NMIX, 0:cc],
                                     start=True, stop=True)
                    rec = mscr.tile([1, CC], F32, tag="rec")
                    nc.vector.reciprocal(out=rec[0:1, 0:cc], in_=sp[0:1, 0:cc])
                    bp = mpsum.tile([NMIX, CC], F32, tag="bp")
                    nc.tensor.matmul(bp[0:NMIX, 0:cc], ones1_20[:], rec[0:1, 0:cc],
                                     start=True, stop=True)
                    nc.vector.tensor_tensor(out=o2[0:20, :], in0=epi[0:NMIX, 0:cc],
                                            in1=bp[0:NMIX, 0:cc], op=ALU.mult)
                    # mu copy (yp1[0:40])
                    nc.vector.tensor_copy(out=o1[0:40, :], in_=yp1[0:40, 0:cc])
                    # rho tanh (yp1[96:116])
                    nc.scalar.activation(out=o1[96:116, :], in_=yp1[96:116, 0:cc],
                                         func=AF.Tanh)
                    # eos sigmoid via tanh (yp1[64:65])
                    teos = mscr.tile([65, CC], F32, tag="teos")
                    nc.scalar.activation(out=teos[64:65, 0:cc], in_=yp1[64:65, 0:cc],
                                         func=AF.Tanh, scale=0.5)
                    nc.vector.tensor_scalar(out=o1[64:65, :], in0=teos[64:65, 0:cc],
                                            scalar1=0.5, scalar2=0.5,
                                            op0=ALU.mult, op1=ALU.add)
                    # s exp (yp2[32:72], split at quadrant boundary)
                    nc.scalar.activation(out=o2[32:64, :], in_=yp2[32:64, 0:cc],
                                         func=AF.Exp)
                    nc.scalar.activation(out=o2[64:72, :], in_=yp2[64:72, 0:cc],
                                         func=AF.Exp)
            nc.gpsimd.dma_start(out=dOUT1[:], in_=OUTS1[:])
            nc.gpsimd.dma_start(out=dOUT2[:], in_=OUTS2[:])

    nc.compile()
    return nc


def _prep_core(inputs, bsl, T):
    x = np.asarray(inputs['input_strokes'], np.float32)
    chars = np.asarray(inputs['input_chars'])
    lens = np.asarray(inputs['input_char_lens'])

    Wx0 = np.asarray(inputs['Wx0'], np.float32)
    Wh0 = np.asarray(inputs['Wh0'], np.float32)
    b0 = np.asarray(inputs['b0'], np.float32)
    Wx1 = np.asarray(inputs['Wx1'], np.float32)
    Wh1 = np.asarray(inputs['Wh1'], np.float32)
    b1 = np.asarray(inputs['b1'], np.float32)
    Wx2 = np.asarray(inputs['Wx2'], np.float32)
    Wh2 = np.asarray(inputs['Wh2'], np.float32)
    b2 = np.asarray(inputs['b2'], np.float32)

    def pack(kblocks):
        nkt = len(kblocks)
        out = np.zeros((128, nkt * 1600), np.float32)
        for k, blk in enumerate(kblocks):
            out[:, k * 1600:(k + 1) * 1600] = blk
        return out.astype(NPBF)

    def kb(rowmap):
        # rowmap: list of (row_start, W rows [n, 1600])
        blk = np.zeros((128, 1600), np.float32)
        for r0, rows in rowmap:
            blk[r0:r0 + rows.shape[0]] = rows
        return blk

    # L1: k-tiles h1 b0-2 + cmbA0 + x12
    W1 = pack([
        kb([(0, Wh0[0:128])]), kb([(0, Wh0[128:256])]), kb([(0, Wh0[256:384])]),
        kb([(0, Wx0[3:76]), (80, b0[None, :]), (96, Wx0[0:3])]),      # cmbA0
        kb([(64, Wh0[384:400])]),                                      # x12
    ])
    # L2: h2 b0-2 + h1 b0-2 + cmbA1
    W2 = pack([
        kb([(0, Wh1[0:128])]), kb([(0, Wh1[128:256])]), kb([(0, Wh1[256:384])]),
        kb([(0, Wx1[76:204])]), kb([(0, Wx1[204:332])]), kb([(0, Wx1[332:460])]),
        kb([(0, Wx1[3:76]), (73, Wx1[0:3]), (76, Wh1[384:400]),
            (92, b1[None, :]), (96, Wx1[460:476])]),                   # cmbA1
    ])
    # L3: h3 b0-2 + h2 b0-2 + cmbA1(w,x only) + x12
    W3 = pack([
        kb([(0, Wh2[0:128])]), kb([(0, Wh2[128:256])]), kb([(0, Wh2[256:384])]),
        kb([(0, Wx2[76:204])]), kb([(0, Wx2[204:332])]), kb([(0, Wx2[332:460])]),
        kb([(0, Wx2[3:76]), (73, Wx2[0:3])]),                          # cmbA1
        kb([(0, Wh2[384:400]), (32, Wx2[460:476]), (96, b2[None, :])]),  # x12
    ])

    PB = np.zeros((128, 9 * 16), np.float32)
    for l in range(3):
        p = np.asarray(inputs['p%d' % l], np.float32)
        for j in range(3):
            pbv = np.zeros((128, 16), np.float32)
            for blk in range(4):
                n = min(128, 400 - 128 * blk)
                pbv[0:n, 4 * blk : 4 * blk + 4] = p[j][128 * blk : 128 * blk + n, None]
            PB[:, (3 * l + j) * 16 : (3 * l + j) * 16 + 16] = pbv

    XT = np.zeros((3, T * NB), np.float32)
    xs = x[bsl]
    for b in range(NB):
        XT[:, b::NB] = xs[b].T
    WATT = np.zeros((128, 4 * 30), np.float32)
    wa = np.asarray(inputs['W_att'], np.float32)
    for k in range(4):
        n = min(128, 400 - 128 * k)
        WATT[0:n, 30 * k : 30 * k + 30] = wa[128 * k : 128 * k + n]
    WATT[96, 90:120] = np.asarray(inputs['b_att'], np.float32)
    V3 = np.concatenate([np.ones(UC), np.arange(UC),
                         np.arange(UC) ** 2]).astype(np.float32)[None, :]
    # OH_aug: rows 0:3 I3 -> cols 73:76 (x), rows 32:48 I16 -> cols 76:92 (h2b3),
    #         rows 64:114 onehot -> cols 0:73 (w)
    OHB = np.zeros((OHROWS, NB * OHCOLS), np.float32)
    for b, gb in enumerate(bsl):
        oh = np.zeros((OHROWS, OHCOLS), np.float32)
        oh[0:3, 73:76] = np.eye(3)
        oh[32:48, 76:92] = np.eye(16)
        ohw = np.zeros((UC, NCHARS), np.float32)
        ohw[np.arange(UC), chars[gb].astype(int)] = 1.0
        ohw[int(lens[gb]):] = 0.0
        oh[64:114, 0:73] = ohw
        OHB[:, OHCOLS * b : OHCOLS * b + OHCOLS] = oh
    # WMDN: per k-tile block [m1(128) | m2(72)]
    wm = np.asarray(inputs['W_mdn'], np.float32)
    bm = np.asarray(inputs['b_mdn'], np.float32)
    wmf = np.zeros((512, 121), np.float32)
    wmf[0:400] = wm
    wmf[3 * 128 + 96] = bm                  # bias via h3all p96 blk3 == 1.0
    m1 = np.zeros((512, 128), np.float32)
    m2 = np.zeros((512, 72), np.float32)
    m1[:, 0:40] = wmf[:, 20:60]             # mu1, mu2
    m1[:, 64:65] = wmf[:, 120:121]          # eos
    m1[:, 96:116] = wmf[:, 100:120]         # rho
    m2[:, 0:20] = wmf[:, 0:20]              # pi
    m2[:, 32:72] = wmf[:, 60:100]           # s1, s2
    WMDN = np.zeros((128, 4 * 200), np.float32)
    for k in range(4):
        WMDN[:, 200 * k : 200 * k + 128] = m1[128 * k : 128 * k + 128]
        WMDN[:, 200 * k + 128 : 200 * k + 200] = m2[128 * k : 128 * k + 128]
    HB = np.zeros((128, 16), np.float32)
    HB[96, 12:16] = 1.0
    CI = np.zeros((128, 12), np.float32)
    CI[80, 0:4] = 1.0    # cmbA0 bias-one row
    CI[92, 4:8] = 1.0    # cmbA1 bias-one row
    CI[96, 8:12] = 1.0   # x12 bias-one row
    return {'W1': W1, 'W2': W2, 'W3': W3, 'PB': PB,
            'XT': XT.astype(NPBF), 'WATT': WATT.astype(NPBF),
            'V3': V3, 'OHB': OHB.astype(NPBF), 'WMDN': WMDN.astype(NPBF),
            'HB': HB.astype(NPBF), 'CI': CI.astype(NPBF)}


def kernel(**inputs):
    x = np.asarray(inputs['input_strokes'])
    B, T, _ = x.shape
    if T not in _CACHE:
        _CACHE[T] = _build_program(T)
    nc = _CACHE[T]
    in_maps = [_prep_core(inputs, list(range(cr * NB, cr * NB + NB)), T)
               for cr in range(NCORES)]
    res = run_bass_kernel_spmd(nc, in_maps, list(range(NCORES)))
    outs = []
    for cr in range(NCORES):
        O1 = res.results[cr]['OUT1'].reshape(128, T, NB)
        O2 = res.results[cr]['OUT2'].reshape(72, T, NB)
        y = np.empty((NB, T, NOUT), np.float32)
        y[..., 0:20] = O2[0:20].transpose(2, 1, 0)
        y[..., 20:60] = O1[0:40].transpose(2, 1, 0)
        y[..., 60:100] = O2[32:72].transpose(2, 1, 0)
        y[..., 100:120] = O1[96:116].transpose(2, 1, 0)
        y[..., 120:121] = O1[64:65].transpose(2, 1, 0)
        outs.append(y)
    return np.concatenate(outs, 0).astype(np.float32)


# revision 16
# speedup vs baseline: 6.9265x; 1.0417x over previous
import sys
sys.path.insert(0, '/opt/trn_rl_repo')
import numpy as np
import ml_dtypes
import concourse.bass as bass
import concourse.bacc as bacc
import concourse.tile as tile
from concourse import mybir
from concourse.bass_utils import run_bass_kernel_spmd

F32 = mybir.dt.float32
BF16 = mybir.dt.float16
NPBF = np.float16
AF = mybir.ActivationFunctionType
ALU = mybir.AluOpType

U = 400        # LSTM units
KATT = 10     # attention gaussians
NCHARS = 73   # alphabet
NMIX = 20     # GMM components
UC = 50       # char positions
NB = 4        # batch per core
NCORES = 8
NOUT = 6 * NMIX + 1  # 121

# m-tiles: per gate [128,128,128,16] -> 16 m-tiles, psum z [128, 64]
MW = [128, 128, 128, 16] * 4
SOFF = [400 * (m // 4) + 128 * (m % 4) for m in range(16)]   # weight-col offsets

# Moving k-tile layouts (partition-write-alignment legal):
#  cmbA0 [128,4]: w(t-1)@0:73 | b0-one@80 | x(t)@96:99
#  cmbA1 [128,4]: w(t)@0:73 | x(t)@73:76 | h2b3(t-1)@76:92 | b1-one@92 | h1b3(t)@96:112
#  x12   [128,4]: h3b3(t-1)@0:16 | h2b3(t)@32:48 | h1b3(t-1)@64:80 | b2-one@96
#  phi_aug [128,4]: x(t)@0:3 | h2b3(t-1)@32:48 | phi@64:114   (window matmul moving)
#  wpA psum [92,4] = OH_aug^T @ phi_aug = w@0:73 | x@73:76 | h2b3@76:92
OHROWS = 114
OHCOLS = 92

_CACHE = {}


def _build_program(T):
    nc = bacc.Bacc("TRN2", target_bir_lowering=False, debug=False, num_devices=NCORES)

    dW1 = nc.dram_tensor("W1", [128, 5 * 1600], BF16, kind="ExternalInput").ap()
    dW2 = nc.dram_tensor("W2", [128, 7 * 1600], BF16, kind="ExternalInput").ap()
    dW3 = nc.dram_tensor("W3", [128, 8 * 1600], BF16, kind="ExternalInput").ap()
    dPB = nc.dram_tensor("PB", [128, 9 * 16], F32, kind="ExternalInput").ap()
    dXT = nc.dram_tensor("XT", [3, T * NB], BF16, kind="ExternalInput").ap()
    dWATT = nc.dram_tensor("WATT", [128, 4 * 30], BF16, kind="ExternalInput").ap()
    dV3 = nc.dram_tensor("V3", [1, 3 * UC], F32, kind="ExternalInput").ap()
    dOHB = nc.dram_tensor("OHB", [OHROWS, NB * OHCOLS], BF16, kind="ExternalInput").ap()
    dWMDN = nc.dram_tensor("WMDN", [128, 4 * 200], BF16, kind="ExternalInput").ap()
    dHB = nc.dram_tensor("HB", [128, 16], BF16, kind="ExternalInput").ap()
    dCI = nc.dram_tensor("CI", [128, 12], BF16, kind="ExternalInput").ap()
    dOUT1 = nc.dram_tensor("OUT1", [128, T * NB], F32, kind="ExternalOutput").ap()
    dOUT2 = nc.dram_tensor("OUT2", [72, T * NB], F32, kind="ExternalOutput").ap()

    with tile.TileContext(nc) as tc:
        with tc.tile_pool(name="statics", bufs=1) as statics, \
             tc.tile_pool(name="states", bufs=1) as states:

            sW1 = statics.tile([128, 5 * 1600], BF16)
            sW2 = statics.tile([128, 7 * 1600], BF16)
            sW3 = statics.tile([128, 8 * 1600], BF16)
            sPB = statics.tile([128, 9 * 16], F32)
            sXT = statics.tile([3, T * NB], BF16)
            sWATT = statics.tile([128, 4 * 30], BF16)
            sV3 = statics.tile([1, 3 * UC], F32)
            sOHB = statics.tile([OHROWS, NB * OHCOLS], BF16)
            sWMDN = statics.tile([128, 4 * 200], BF16)
            sHB = statics.tile([128, 16], BF16)
            for dst, src in [(sW1, dW1), (sW2, dW2), (sW3, dW3), (sPB, dPB),
                             (sXT, dXT), (sWATT, dWATT), (sV3, dV3),
                             (sOHB, dOHB), (sWMDN, dWMDN), (sHB, dHB)]:
                nc.gpsimd.dma_start(out=dst[:], in_=src[:])

            h3all = states.tile([128, T * 16], BF16)
            OUTS1 = states.tile([128, T * NB], F32)
            OUTS2 = states.tile([72, T * NB], F32)
            h = [states.tile([128, 16], BF16, name=f"h{i}") for i in range(3)]
            # c stored duplicated [c|c] so the i,f peephole ops fuse to [128,32]
            c = [states.tile([128, 32], F32, name=f"c{i}") for i in range(3)]
            cmbA0 = states.tile([128, NB], BF16, name="cmbA0")
            cmbA1 = states.tile([128, NB], BF16, name="cmbA1")
            x12 = states.tile([128, NB], BF16, name="x12")
            phi_aug = states.tile([128, NB], BF16, name="phi_aug")
            # DMA-init combo tiles: zeros + bias-one rows (avoids unaligned writes)
            nc.gpsimd.dma_start(out=cmbA0[:], in_=dCI[:, 0:4])
            nc.gpsimd.dma_start(out=cmbA1[:], in_=dCI[:, 4:8])
            nc.gpsimd.dma_start(out=x12[:], in_=dCI[:, 8:12])
            kap = states.tile([1, 4 * KATT], F32)  # (b, k)
            for tl in c:
                nc.vector.memset(tl[:], 0.0)
            nc.vector.memset(phi_aug[:], 0.0)
            nc.vector.memset(kap[:], 0.0)
            nc.vector.memset(h3all[:], 0.0)
            # h init: zeros except bias constant 1.0 at (p96, blk3 cols)
            for tl in h:
                nc.vector.tensor_copy(out=tl[:], in_=sHB[:])
            nc.vector.memset(h3all[96:128, :], 1.0)

            with tc.tile_pool(name="psum", bufs=1, space="PSUM") as psum, \
                 tc.tile_pool(name="scratch", bufs=2) as scratch:

                z = [psum.tile([128, 64], F32, name=f"z{i}") for i in range(3)]
                attp = psum.tile([1, NB * 30], F32)
                argp = psum.tile([UC, NB * KATT], F32)
                wpA = psum.tile([OHCOLS, NB], F32)
                for tl in z:
                    nc.vector.memset(tl[:], 0.0)

                _sc = {}
                for _t in ["tg", "m1", "m2", "pc2", "zo_", "to", "og", "tcn",
                           "hb3"]:
                    _sc[_t] = scratch.tile([128, 16], F32, tag=_t, name=_t)
                for _t in ["pcif", "zif_", "tif", "igfg"]:
                    _sc[_t] = scratch.tile([128, 32], F32, tag=_t, name=_t)
                for _t, _shp, _dt in [("E", [1, NB * 30], F32),
                                      ("kap2", [1, NB * KATT], F32),
                                      ("bk", [1, NB * KATT], F32),
                                      ("A_", [1, NB * KATT], F32),
                                      ("bk2", [1, NB * KATT], F32),
                                      ("B_", [1, NB * KATT], F32),
                                      ("C_", [1, NB * KATT], F32),
                                      ("P", [UC, NB * KATT], BF16)]:
                    _sc[_t] = scratch.tile(_shp, _dt, tag=_t, name=_t)

                def layer_mms(l, sW, movs, first, last):
                    # start=True clears has_written for the WHOLE psum bank,
                    # so a step may carry exactly ONE start per z tile: the
                    # very first matmul (bank clear -> every region's first
                    # touch overwrites, later ones accumulate). This lets the
                    # recurrence-independent partials issue early without
                    # corrupting open accumulations in the same bank.
                    for m in range(16):
                        w0, mw = int(SOFF[m]), MW[m]
                        for ki, (mov, kidx) in enumerate(movs):
                            kp = mov.shape[0]
                            nc.tensor.matmul(
                                z[l][0:mw, 4 * m : 4 * m + 4],
                                sW[0:kp, kidx * 1600 + w0 : kidx * 1600 + w0 + mw],
                                mov,
                                start=(first and m == 0 and ki == 0),
                                stop=(last and ki == len(movs) - 1),
                                skip_group_check=True,
                            )

                def st(tag):
                    return _sc[tag]

                def cell(l, t):
                    zt = z[l]
                    zif = zt[:, 0:32]
                    zg, zo = zt[:, 32:48], zt[:, 48:64]
                    p01 = sPB[:, 48 * l : 48 * l + 32]
                    p2 = sPB[:, 48 * l + 32 : 48 * l + 48]
                    ct = c[l]        # [128, 32] duplicated [c|c]
                    cn = ct[:, 0:16]
                    pcif = st("pcif")
                    nc.gpsimd.tensor_tensor(out=pcif[:], in0=p01, in1=ct[:],
                                            op=ALU.mult)
                    zif_ = st("zif_")
                    nc.vector.tensor_tensor(out=zif_[:], in0=zif, in1=pcif[:],
                                            op=ALU.add)
                    tif = st("tif")
                    nc.scalar.activation(out=tif[:], in_=zif_[:], func=AF.Tanh,
                                         scale=0.5)
                    igfg = st("igfg")
                    nc.vector.tensor_scalar(out=igfg[:], in0=tif[:], scalar1=0.5,
                                            scalar2=0.5, op0=ALU.mult, op1=ALU.add)
                    tg = st("tg")
                    nc.scalar.activation(out=tg[:], in_=zg, func=AF.Tanh)
                    m1 = st("m1")
                    nc.vector.tensor_tensor(out=m1[:], in0=igfg[:, 0:16], in1=tg[:],
                                            op=ALU.mult)
                    m2 = st("m2")
                    nc.gpsimd.tensor_tensor(out=m2[:], in0=igfg[:, 16:32], in1=cn,
                                            op=ALU.mult)
                    nc.vector.tensor_tensor(out=cn, in0=m1[:], in1=m2[:], op=ALU.add)
                    # duplicate halves for next step's fused peephole (off-chain)
                    nc.gpsimd.tensor_copy(out=ct[:, 16:32], in_=cn)
                    pc2 = st("pc2")
                    nc.vector.tensor_tensor(out=pc2[:], in0=p2, in1=cn, op=ALU.mult)
                    zo_ = st("zo_")
                    nc.vector.tensor_tensor(out=zo_[:], in0=zo, in1=pc2[:], op=ALU.add)
                    to = st("to")
                    nc.scalar.activation(out=to[:], in_=zo_[:], func=AF.Tanh, scale=0.5)
                    og = st("og")
                    nc.vector.tensor_scalar(out=og[:], in0=to[:], scalar1=0.5,
                                            scalar2=0.5, op0=ALU.mult, op1=ALU.add)
                    tcn = st("tcn")
                    nc.scalar.activation(out=tcn[:], in_=cn, func=AF.Tanh)
                    # h update
                    nc.vector.tensor_tensor(out=h[l][:, 0:12], in0=og[:, 0:12],
                                            in1=tcn[:, 0:12], op=ALU.mult)
                    if l != 1:
                        nc.vector.tensor_tensor(out=h[l][0:96, 12:16],
                                                in0=og[0:96, 12:16],
                                                in1=tcn[0:96, 12:16], op=ALU.mult)
                    # blk3 (units 384:400) products into combo tiles
                    hb3 = st("hb3")
                    nc.gpsimd.tensor_tensor(out=hb3[0:16, 0:4], in0=og[0:16, 12:16],
                                            in1=tcn[0:16, 12:16], op=ALU.mult)
                    if l == 0:
                        nc.gpsimd.tensor_copy(out=cmbA1[96:112, :], in_=hb3[0:16, 0:4])
                        nc.gpsimd.tensor_copy(out=x12[64:80, :], in_=hb3[0:16, 0:4])
                    elif l == 1:
                        nc.gpsimd.tensor_copy(out=phi_aug[32:48, :], in_=hb3[0:16, 0:4])
                        nc.gpsimd.tensor_copy(out=x12[32:48, :], in_=hb3[0:16, 0:4])
                    else:
                        nc.gpsimd.tensor_copy(out=x12[0:16, :], in_=hb3[0:16, 0:4])
                        nc.vector.tensor_copy(
                            out=h3all[:, bass.DynSlice(t * 16, 12)],
                            in_=h[2][:, 0:12])
                        nc.vector.tensor_copy(
                            out=h3all[0:96, bass.DynSlice(t * 16 + 12, 4)],
                            in_=h[2][0:96, 12:16])

                def attention():
                    for b in range(NB):
                        for k in range(4):
                            nc.tensor.matmul(
                                attp[0:1, 30 * b : 30 * b + 30],
                                h[0][:, 4 * k + b : 4 * k + b + 1],
                                sWATT[:, 30 * k : 30 * k + 30],
                                start=(k == 0), stop=(k == 3),
                            )
                    E = _sc["E"]
                    nc.scalar.activation(out=E[:], in_=attp[:], func=AF.Exp)
                    Ev = E[0:1, :].rearrange("p (b x) -> p b x", b=NB)
                    av = attp[0:1, :].rearrange("p (b x) -> p b x", b=NB)
                    kapv = kap[0:1, :].rearrange("p (b k) -> p b k", b=NB)
                    kap2 = _sc["kap2"]
                    k2v = kap2[0:1, :].rearrange("p (b k) -> p b k", b=NB)
                    nc.vector.tensor_tensor(out=k2v, in0=kapv, in1=Ev[:, :, 20:30],
                                            op=ALU.add)
                    nc.gpsimd.tensor_copy(out=kap[:], in_=kap2[:])
                    bk = _sc["bk"]
                    bkv = bk[0:1, :].rearrange("p (b k) -> p b k", b=NB)
                    nc.vector.tensor_tensor(out=bkv, in0=Ev[:, :, 10:20], in1=k2v,
                                            op=ALU.mult)
                    A_ = _sc["A_"]
                    Av = A_[0:1, :].rearrange("p (b k) -> p b k", b=NB)
                    bk2 = _sc["bk2"]
                    bk2v = bk2[0:1, :].rearrange("p (b k) -> p b k", b=NB)
                    nc.vector.tensor_tensor(out=bk2v, in0=bkv, in1=k2v, op=ALU.mult)
                    nc.vector.tensor_tensor(out=Av, in0=av[:, :, 0:10], in1=bk2v,
                                            op=ALU.subtract)
                    B_ = _sc["B_"]
                    nc.vector.tensor_scalar(out=B_[:], in0=bk[:], scalar1=2.0,
                                            scalar2=None, op0=ALU.mult)
                    C_ = _sc["C_"]
                    Cv = C_[0:1, :].rearrange("p (b k) -> p b k", b=NB)
                    nc.vector.tensor_scalar(out=Cv, in0=Ev[:, :, 10:20], scalar1=-1.0,
                                            scalar2=None, op0=ALU.mult)
                    # arg[u,(b,k)] = A + u*B + u^2*C  via 3 accumulating K=1 matmuls
                    nc.tensor.matmul(argp[:], sV3[0:1, 0:UC], A_[:],
                                     start=True, stop=False)
                    nc.tensor.matmul(argp[:], sV3[0:1, UC:2 * UC], B_[:],
                                     start=False, stop=False)
                    nc.tensor.matmul(argp[:], sV3[0:1, 2 * UC:3 * UC], C_[:],
                                     start=False, stop=True)
                    P = _sc["P"]
                    nc.scalar.activation(out=P[:], in_=argp[:], func=AF.Exp)
                    Pv = P[:, :].rearrange("p (b k) -> p b k", b=NB)
                    with nc.allow_low_precision(reason="phi: sum of 10 pos bf16"):
                        nc.vector.tensor_reduce(out=phi_aug[64:114, :], in_=Pv,
                                                axis=mybir.AxisListType.X, op=ALU.add)
                    # wpA = OH_aug^T @ phi_aug : [w | x | h2b3] assembled in psum
                    for b in range(NB):
                        nc.tensor.matmul(
                            wpA[:, b : b + 1],
                            sOHB[:, OHCOLS * b : OHCOLS * b + OHCOLS],
                            phi_aug[0:OHROWS, b : b + 1],
                            start=True, stop=True,
                        )
                    nc.vector.tensor_copy(out=cmbA1[0:92, :], in_=wpA[0:92, :])
                    nc.vector.tensor_copy(out=cmbA0[0:73, :], in_=wpA[0:73, :])

                import os as _os
                import contextlib as _cl
                _pyloop = bool(_os.environ.get("KPYLOOP"))

                def _loop():
                    if _pyloop:
                        return _cl.nullcontext(range(T))
                    return tc.For_i(0, T)

                with _loop() as _ts:
                    _titer = _ts if _pyloop else [_ts]
                    for t in _titer:
                        nc.gpsimd.tensor_copy(out=cmbA0[96:99, :],
                                              in_=sXT[0:3, bass.ts(t, NB)])
                        nc.gpsimd.tensor_copy(out=phi_aug[0:3, :],
                                              in_=sXT[0:3, bass.ts(t, NB)])
                        layer_mms(0, sW1, [(h[0][:, 0:4], 0), (h[0][:, 4:8], 1),
                                           (h[0][:, 8:12], 2), (cmbA0[:], 3),
                                           (x12[:], 4)], first=True, last=True)
                        # recurrence-independent partials of z2/z3 issue now so
                        # the PE stays busy under the cell-0 vector chain
                        layer_mms(1, sW2, [(h[1][:, 0:4], 0), (h[1][:, 4:8], 1),
                                           (h[1][:, 8:12], 2)],
                                  first=True, last=False)
                        layer_mms(2, sW3, [(h[2][:, 0:4], 0), (h[2][:, 4:8], 1),
                                           (h[2][:, 8:12], 2)],
                                  first=True, last=False)
                        cell(0, t)
                        attention()
                        layer_mms(1, sW2, [(h[0][:, 0:4], 3), (h[0][:, 4:8], 4),
                                           (h[0][:, 8:12], 5), (cmbA1[:], 6)],
                                  first=False, last=True)
                        cell(1, t)
                        layer_mms(2, sW3, [(h[1][:, 0:4], 3), (h[1][:, 4:8], 4),
                                           (h[1][:, 8:12], 5), (cmbA1[:], 6),
                                           (x12[:], 7)],
                                  first=False, last=True)
                        cell(2, t)

            # ---- MDN head ----
            # Y1 rows: mu @0:40, eos @64, rho @96:116 ; Y2 rows: pi @0:20, s @32:72
            with tc.tile_pool(name="mpsum", bufs=2, space="PSUM") as mpsum, \
                 tc.tile_pool(name="mscr", bufs=2) as mscr, \
                 tc.tile_pool(name="mones", bufs=1) as mones:
                ones20 = mones.tile([NMIX, 1], F32)
                nc.vector.memset(ones20[:], 1.0)
                ones1_20 = mones.tile([1, NMIX], F32)
                nc.vector.memset(ones1_20[:], 1.0)
                h3v = h3all[:, :].rearrange("p (t x) -> p t x", t=T)
                CC = min(400, T * NB)
                TC = CC // NB
                for ch in range((T + TC - 1) // TC):
                    t0 = TC * ch
                    tn = min(TC, T - t0)
                    cc = tn * NB
                    yp1 = mpsum.tile([128, CC], F32, tag="yp1")
                    yp2 = mpsum.tile([72, CC], F32, tag="yp2")
                    for k in range(4):
                        nc.tensor.matmul(
                            yp1[0:128, 0:cc],
                            sWMDN[:, 200 * k : 200 * k + 128],
                            h3v[:, t0 : t0 + tn, 4 * k : 4 * k + 4],
                            start=(k == 0), stop=(k == 3))
                    for k in range(4):
                        nc.tensor.matmul(
                            yp2[0:72, 0:cc],
                            sWMDN[:, 200 * k + 128 : 200 * k + 200],
                            h3v[:, t0 : t0 + tn, 4 * k : 4 * k + 4],
                            start=(k == 0), stop=(k == 3))
                    o1 = OUTS1[:, NB * t0 : NB * t0 + cc]
                    o2 = OUTS2[:, NB * t0 : NB * t0 + cc]
                    # pi softmax (pi lives at yp2[0:20])
                    epi = mscr.tile([NMIX, CC], F32, tag="epi")
                    nc.scalar.activation(out=epi[0:NMIX, 0:cc], in_=yp2[0:NMIX, 0:cc],
                                         func=AF.Exp)
                    sp = mpsum.tile([1, CC], F32, tag="sp")
                    nc.tensor.matmul(sp[0:1, 0:cc], ones20[:], epi[0:NMIX, 0:cc],
                                     start=True, stop=True)
                    rec = mscr.tile([1, CC], F32, tag="rec")
                    nc.vector.reciprocal(out=rec[0:1, 0:cc], in_=sp[0:1, 0:cc])
                    bp = mpsum.tile([NMIX, CC], F32, tag="bp")
                    nc.tensor.matmul(bp[0:NMIX, 0:cc], ones1_20[:], rec[0:1, 0:cc],
                                     start=True, stop=True)
                    nc.vector.tensor_tensor(out=o2[0:20, :], in0=epi[0:NMIX, 0:cc],
                                            in1=bp[0:NMIX, 0:cc], op=ALU.mult)
                    # mu copy (yp1[0:40])
                    nc.vector.tensor_copy(out=o1[0:40, :], in_=yp1[0:40, 0:cc])
                    # rho tanh (yp1[96:116])
                    nc.scalar.activation(out=o1[96:116, :], in_=yp1[96:116, 0:cc],
                                         func=AF.Tanh)
                    # eos sigmoid via tanh (yp1[64:65])
                    teos = mscr.tile([65, CC], F32, tag="teos")
                    nc.scalar.activation(out=teos[64:65, 0:cc], in_=yp1[64:65, 0:cc],
                                         func=AF.Tanh, scale=0.5)
                    nc.vector.tensor_scalar(out=o1[64:65, :], in0=teos[64:65, 0:cc],
                                            scalar1=0.5, scalar2=0.5,
                                            op0=ALU.mult, op1=ALU.add)
                    # s exp (yp2[32:72], split at quadrant boundary)
                    nc.scalar.activation(out=o2[32:64, :], in_=yp2[32:64, 0:cc],
                                         func=AF.Exp)
                    nc.scalar.activation(out=o2[64:72, :], in_=yp2[64:72, 0:cc],
                                         func=AF.Exp)
            nc.gpsimd.dma_start(out=dOUT1[:], in_=OUTS1[:])
            nc.gpsimd.dma_start(out=dOUT2[:], in_=OUTS2[:])

    nc.compile()
    return nc


def _prep_core(inputs, bsl, T):
    x = np.asarray(inputs['input_strokes'], np.float32)
    chars = np.asarray(inputs['input_chars'])
    lens = np.asarray(inputs['input_char_lens'])

    Wx0 = np.asarray(inputs['Wx0'], np.float32)
    Wh0 = np.asarray(inputs['Wh0'], np.float32)
    b0 = np.asarray(inputs['b0'], np.float32)
    Wx1 = np.asarray(inputs['Wx1'], np.float32)
    Wh1 = np.asarray(inputs['Wh1'], np.float32)
    b1 = np.asarray(inputs['b1'], np.float32)
    Wx2 = np.asarray(inputs['Wx2'], np.float32)
    Wh2 = np.asarray(inputs['Wh2'], np.float32)
    b2 = np.asarray(inputs['b2'], np.float32)

    def pack(kblocks):
        nkt = len(kblocks)
        out = np.zeros((128, nkt * 1600), np.float32)
        for k, blk in enumerate(kblocks):
            out[:, k * 1600:(k + 1) * 1600] = blk
        return out.astype(NPBF)

    def kb(rowmap):
        # rowmap: list of (row_start, W rows [n, 1600])
        blk = np.zeros((128, 1600), np.float32)
        for r0, rows in rowmap:
            blk[r0:r0 + rows.shape[0]] = rows
        return blk

    # L1: k-tiles h1 b0-2 + cmbA0 + x12
    W1 = pack([
        kb([(0, Wh0[0:128])]), kb([(0, Wh0[128:256])]), kb([(0, Wh0[256:384])]),
        kb([(0, Wx0[3:76]), (80, b0[None, :]), (96, Wx0[0:3])]),      # cmbA0
        kb([(64, Wh0[384:400])]),                                      # x12
    ])
    # L2: h2 b0-2 + h1 b0-2 + cmbA1
    W2 = pack([
        kb([(0, Wh1[0:128])]), kb([(0, Wh1[128:256])]), kb([(0, Wh1[256:384])]),
        kb([(0, Wx1[76:204])]), kb([(0, Wx1[204:332])]), kb([(0, Wx1[332:460])]),
        kb([(0, Wx1[3:76]), (73, Wx1[0:3]), (76, Wh1[384:400]),
            (92, b1[None, :]), (96, Wx1[460:476])]),                   # cmbA1
    ])
    # L3: h3 b0-2 + h2 b0-2 + cmbA1(w,x only) + x12
    W3 = pack([
        kb([(0, Wh2[0:128])]), kb([(0, Wh2[128:256])]), kb([(0, Wh2[256:384])]),
        kb([(0, Wx2[76:204])]), kb([(0, Wx2[204:332])]), kb([(0, Wx2[332:460])]),
        kb([(0, Wx2[3:76]), (73, Wx2[0:3])]),                          # cmbA1
        kb([(0, Wh2[384:400]), (32, Wx2[460:476]), (96, b2[None, :])]),  # x12
    ])

    PB = np.zeros((128, 9 * 16), np.float32)
    for l in range(3):
        p = np.asarray(inputs['p%d' % l], np.float32)
        for j in range(3):
            pbv = np.zeros((128, 16), np.float32)
            for blk in range(4):
                n = min(128, 400 - 128 * blk)
                pbv[0:n, 4 * blk : 4 * blk + 4] = p[j][128 * blk : 128 * blk + n, None]
            PB[:, (3 * l + j) * 16 : (3 * l + j) * 16 + 16] = pbv

    XT = np.zeros((3, T * NB), np.float32)
    xs = x[bsl]
    for b in range(NB):
        XT[:, b::NB] = xs[b].T
    WATT = np.zeros((128, 4 * 30), np.float32)
    wa = np.asarray(inputs['W_att'], np.float32)
    for k in range(4):
        n = min(128, 400 - 128 * k)
        WATT[0:n, 30 * k : 30 * k + 30] = wa[128 * k : 128 * k + n]
    WATT[96, 90:120] = np.asarray(inputs['b_att'], np.float32)
    V3 = np.concatenate([np.ones(UC), np.arange(UC),
                         np.arange(UC) ** 2]).astype(np.float32)[None, :]
    # OH_aug: rows 0:3 I3 -> cols 73:76 (x), rows 32:48 I16 -> cols 76:92 (h2b3),
    #         rows 64:114 onehot -> cols 0:73 (w)
    OHB = np.zeros((OHROWS, NB * OHCOLS), np.float32)
    for b, gb in enumerate(bsl):
        oh = np.zeros((OHROWS, OHCOLS), np.float32)
        oh[0:3, 73:76] = np.eye(3)
        oh[32:48, 76:92] = np.eye(16)
        ohw = np.zeros((UC, NCHARS), np.float32)
        ohw[np.arange(UC), chars[gb].astype(int)] = 1.0
        ohw[int(lens[gb]):] = 0.0
        oh[64:114, 0:73] = ohw
        OHB[:, OHCOLS * b : OHCOLS * b + OHCOLS] = oh
    # WMDN: per k-tile block [m1(128) | m2(72)]
    wm = np.asarray(inputs['W_mdn'], np.float32)
    bm = np.asarray(inputs['b_mdn'], np.float32)
    wmf = np.zeros((512, 121), np.float32)
    wmf[0:400] = wm
    wmf[3 * 128 + 96] = bm                  # bias via h3all p96 blk3 == 1.0
    m1 = np.zeros((512, 128), np.float32)
    m2 = np.zeros((512, 72), np.float32)
    m1[:, 0:40] = wmf[:, 20:60]             # mu1, mu2
    m1[:, 64:65] = wmf[:, 120:121]          # eos
    m1[:, 96:116] = wmf[:, 100:120]         # rho
    m2[:, 0:20] = wmf[:, 0:20]              # pi
    m2[:, 32:72] = wmf[:, 60:100]           # s1, s2
    WMDN = np.zeros((128, 4 * 200), np.float32)
    for k in range(4):
        WMDN[:, 200 * k : 200 * k + 128] = m1[128 * k : 128 * k + 128]
        WMDN[:, 200 * k + 128 : 200 * k + 200] = m2[128 * k : 128 * k + 128]
    HB = np.zeros((128, 16), np.float32)
    HB[96, 12:16] = 1.0
    CI = np.zeros((128, 12), np.float32)
    CI[80, 0:4] = 1.0    # cmbA0 bias-one row
    CI[92, 4:8] = 1.0    # cmbA1 bias-one row
    CI[96, 8:12] = 1.0   # x12 bias-one row
    return {'W1': W1, 'W2': W2, 'W3': W3, 'PB': PB,
            'XT': XT.astype(NPBF), 'WATT': WATT.astype(NPBF),
            'V3': V3, 'OHB': OHB.astype(NPBF), 'WMDN': WMDN.astype(NPBF),
            'HB': HB.astype(NPBF), 'CI': CI.astype(NPBF)}


def kernel(**inputs):
    x = np.asarray(inputs['input_strokes'])
    B, T, _ = x.shape
    if T not in _CACHE:
        _CACHE[T] = _build_program(T)
    nc = _CACHE[T]
    in_maps = [_prep_core(inputs, list(range(cr * NB, cr * NB + NB)), T)
               for cr in range(NCORES)]
    res = run_bass_kernel_spmd(nc, in_maps, list(range(NCORES)))
    outs = []
    for cr in range(NCORES):
        O1 = res.results[cr]['OUT1'].reshape(128, T, NB)
        O2 = res.results[cr]['OUT2'].reshape(72, T, NB)
        y = np.empty((NB, T, NOUT), np.float32)
        y[..., 0:20] = O2[0:20].transpose(2, 1, 0)
        y[..., 20:60] = O1[0:40].transpose(2, 1, 0)
        y[..., 60:100] = O2[32:72].transpose(2, 1, 0)
        y[..., 100:120] = O1[96:116].transpose(2, 1, 0)
        y[..., 120:121] = O1[64:65].transpose(2, 1, 0)
        outs.append(y)
    return np.concatenate(outs, 0).astype(np.float32)


# revision 21
# speedup vs baseline: 6.9304x; 1.0006x over previous
import sys
sys.path.insert(0, '/opt/trn_rl_repo')
import numpy as np
import ml_dtypes
import concourse.bass as bass
import concourse.bacc as bacc
import concourse.tile as tile
from concourse import mybir
from concourse.bass_utils import run_bass_kernel_spmd

F32 = mybir.dt.float32
BF16 = mybir.dt.float16
NPBF = np.float16
AF = mybir.ActivationFunctionType
ALU = mybir.AluOpType

U = 400        # LSTM units
KATT = 10     # attention gaussians
NCHARS = 73   # alphabet
NMIX = 20     # GMM components
UC = 50       # char positions
NB = 4        # batch per core
NCORES = 8
NOUT = 6 * NMIX + 1  # 121

# m-tiles: per gate [128,128,128,16] -> 16 m-tiles, psum z [128, 64]
MW = [128, 128, 128, 16] * 4
SOFF = [400 * (m // 4) + 128 * (m % 4) for m in range(16)]   # weight-col offsets

# Moving k-tile layouts (partition-write-alignment legal):
#  cmbA0 [128,4]: w(t-1)@0:73 | b0-one@80 | x(t)@96:99
#  cmbA1 [128,4]: w(t)@0:73 | x(t)@73:76 | h2b3(t-1)@76:92 | b1-one@92 | h1b3(t)@96:112
#  x12   [128,4]: h3b3(t-1)@0:16 | h2b3(t)@32:48 | h1b3(t-1)@64:80 | b2-one@96
#  phi_aug [128,4]: x(t)@0:3 | h2b3(t-1)@32:48 | phi@64:114   (window matmul moving)
#  wpA psum [92,4] = OH_aug^T @ phi_aug = w@0:73 | x@73:76 | h2b3@76:92
OHROWS = 114
OHCOLS = 92

_CACHE = {}


def _build_program(T):
    nc = bacc.Bacc("TRN2", target_bir_lowering=False, debug=False, num_devices=NCORES)

    dW1 = nc.dram_tensor("W1", [128, 5 * 1600], BF16, kind="ExternalInput").ap()
    dW2 = nc.dram_tensor("W2", [128, 7 * 1600], BF16, kind="ExternalInput").ap()
    dW3 = nc.dram_tensor("W3", [128, 8 * 1600], BF16, kind="ExternalInput").ap()
    dPB = nc.dram_tensor("PB", [128, 9 * 16], F32, kind="ExternalInput").ap()
    dXT = nc.dram_tensor("XT", [3, T * NB], BF16, kind="ExternalInput").ap()
    dWATT = nc.dram_tensor("WATT", [128, 4 * 30], BF16, kind="ExternalInput").ap()
    dV3 = nc.dram_tensor("V3", [1, 3 * UC], F32, kind="ExternalInput").ap()
    dOHB = nc.dram_tensor("OHB", [OHROWS, NB * OHCOLS], BF16, kind="ExternalInput").ap()
    dWMDN = nc.dram_tensor("WMDN", [128, 4 * 200], BF16, kind="ExternalInput").ap()
    dHB = nc.dram_tensor("HB", [128, 16], BF16, kind="ExternalInput").ap()
    dCI = nc.dram_tensor("CI", [128, 12], BF16, kind="ExternalInput").ap()
    dOUT1 = nc.dram_tensor("OUT1", [128, T * NB], F32, kind="ExternalOutput").ap()
    dOUT2 = nc.dram_tensor("OUT2", [72, T * NB], F32, kind="ExternalOutput").ap()

    with tile.TileContext(nc) as tc:
        with tc.tile_pool(name="statics", bufs=1) as statics, \
             tc.tile_pool(name="states", bufs=1) as states:

            sW1 = statics.tile([128, 5 * 1600], BF16)
            sW2 = statics.tile([128, 7 * 1600], BF16)
            sW3 = statics.tile([128, 8 * 1600], BF16)
            sPB = statics.tile([128, 9 * 16], F32)
            sXT = statics.tile([3, T * NB], BF16)
            sWATT = statics.tile([128, 4 * 30], BF16)
            sV3 = statics.tile([1, 3 * UC], F32)
            sOHB = statics.tile([OHROWS, NB * OHCOLS], BF16)
            sWMDN = statics.tile([128, 4 * 200], BF16)
            sHB = statics.tile([128, 16], BF16)
            for dst, src in [(sW1, dW1), (sW2, dW2), (sW3, dW3), (sPB, dPB),
                             (sXT, dXT), (sWATT, dWATT), (sV3, dV3),
                             (sOHB, dOHB), (sWMDN, dWMDN), (sHB, dHB)]:
                nc.gpsimd.dma_start(out=dst[:], in_=src[:])

            h3all = states.tile([128, T * 16], BF16)
            OUTS1 = states.tile([128, T * NB], F32)
            OUTS2 = states.tile([72, T * NB], F32)
            h = [states.tile([128, 16], BF16, name=f"h{i}") for i in range(3)]
            # c stored duplicated [c|c] so the i,f peephole ops fuse to [128,32]
            c = [states.tile([128, 32], F32, name=f"c{i}") for i in range(3)]
            cmbA0 = states.tile([128, NB], BF16, name="cmbA0")
            cmbA1 = states.tile([128, NB], BF16, name="cmbA1")
            x12 = states.tile([128, NB], BF16, name="x12")
            phi_aug = states.tile([128, NB], BF16, name="phi_aug")
            # DMA-init combo tiles: zeros + bias-one rows (avoids unaligned writes)
            nc.gpsimd.dma_start(out=cmbA0[:], in_=dCI[:, 0:4])
            nc.gpsimd.dma_start(out=cmbA1[:], in_=dCI[:, 4:8])
            nc.gpsimd.dma_start(out=x12[:], in_=dCI[:, 8:12])
            kap = states.tile([1, 4 * KATT], F32)  # (b, k)
            for tl in c:
                nc.vector.memset(tl[:], 0.0)
            nc.vector.memset(phi_aug[:], 0.0)
            nc.vector.memset(kap[:], 0.0)
            nc.vector.memset(h3all[:], 0.0)
            # h init: zeros except bias constant 1.0 at (p96, blk3 cols)
            for tl in h:
                nc.vector.tensor_copy(out=tl[:], in_=sHB[:])
            nc.vector.memset(h3all[96:128, :], 1.0)

            with tc.tile_pool(name="psum", bufs=1, space="PSUM") as psum, \
                 tc.tile_pool(name="scratch", bufs=2) as scratch:

                z = [psum.tile([128, 64], F32, name=f"z{i}") for i in range(3)]
                attp = psum.tile([1, NB * 30], F32)
                argp = psum.tile([UC, NB * KATT], F32)
                wpA = psum.tile([OHCOLS, NB], F32)
                for tl in z:
                    nc.vector.memset(tl[:], 0.0)

                _sc = {}
                for _t in ["tg", "m1", "m2", "pc2", "zo_", "to", "og", "tcn",
                           "hb3"]:
                    _sc[_t] = scratch.tile([128, 16], F32, tag=_t, name=_t)
                for _t in ["pcif", "zif_", "tif", "igfg"]:
                    _sc[_t] = scratch.tile([128, 32], F32, tag=_t, name=_t)
                for _t, _shp, _dt in [("E", [1, NB * 30], F32),
                                      ("kap2", [1, NB * KATT], F32),
                                      ("bk", [1, NB * KATT], F32),
                                      ("A_", [1, NB * KATT], F32),
                                      ("bk2", [1, NB * KATT], F32),
                                      ("B_", [1, NB * KATT], F32),
                                      ("C_", [1, NB * KATT], F32),
                                      ("P", [UC, NB * KATT], BF16)]:
                    _sc[_t] = scratch.tile(_shp, _dt, tag=_t, name=_t)

                def layer_mms(l, sW, movs, first, last):
                    # start=True clears has_written for the WHOLE psum bank,
                    # so a step may carry exactly ONE start per z tile: the
                    # very first matmul (bank clear -> every region's first
                    # touch overwrites, later ones accumulate). This lets the
                    # recurrence-independent partials issue early without
                    # corrupting open accumulations in the same bank.
                    for m in range(16):
                        w0, mw = int(SOFF[m]), MW[m]
                        for ki, (mov, kidx) in enumerate(movs):
                            kp = mov.shape[0]
                            nc.tensor.matmul(
                                z[l][0:mw, 4 * m : 4 * m + 4],
                                sW[0:kp, kidx * 1600 + w0 : kidx * 1600 + w0 + mw],
                                mov,
                                start=(first and m == 0 and ki == 0),
                                stop=(last and ki == len(movs) - 1),
                                skip_group_check=True,
                            )

                def st(tag):
                    return _sc[tag]

                def cell(l, t):
                    zt = z[l]
                    zif = zt[:, 0:32]
                    zg, zo = zt[:, 32:48], zt[:, 48:64]
                    p01 = sPB[:, 48 * l : 48 * l + 32]
                    p2 = sPB[:, 48 * l + 32 : 48 * l + 48]
                    ct = c[l]        # [128, 32] duplicated [c|c]
                    cn = ct[:, 0:16]
                    pcif = st("pcif")
                    nc.gpsimd.tensor_tensor(out=pcif[:], in0=p01, in1=ct[:],
                                            op=ALU.mult)
                    zif_ = st("zif_")
                    nc.vector.tensor_tensor(out=zif_[:], in0=zif, in1=pcif[:],
                                            op=ALU.add)
                    tif = st("tif")
                    nc.scalar.activation(out=tif[:], in_=zif_[:], func=AF.Tanh,
                                         scale=0.5)
                    igfg = st("igfg")
                    nc.vector.tensor_scalar(out=igfg[:], in0=tif[:], scalar1=0.5,
                                            scalar2=0.5, op0=ALU.mult, op1=ALU.add)
                    tg = st("tg")
                    nc.scalar.activation(out=tg[:], in_=zg, func=AF.Tanh)
                    m1 = st("m1")
                    nc.vector.tensor_tensor(out=m1[:], in0=igfg[:, 0:16], in1=tg[:],
                                            op=ALU.mult)
                    m2 = st("m2")
                    nc.gpsimd.tensor_tensor(out=m2[:], in0=igfg[:, 16:32], in1=cn,
                                            op=ALU.mult)
                    nc.vector.tensor_tensor(out=cn, in0=m1[:], in1=m2[:], op=ALU.add)
                    # duplicate halves for next step's fused peephole (off-chain)
                    nc.gpsimd.tensor_copy(out=ct[:, 16:32], in_=cn)
                    pc2 = st("pc2")
                    nc.vector.tensor_tensor(out=pc2[:], in0=p2, in1=cn, op=ALU.mult)
                    zo_ = st("zo_")
                    nc.vector.tensor_tensor(out=zo_[:], in0=zo, in1=pc2[:], op=ALU.add)
                    to = st("to")
                    nc.scalar.activation(out=to[:], in_=zo_[:], func=AF.Tanh, scale=0.5)
                    og = st("og")
                    nc.vector.tensor_scalar(out=og[:], in0=to[:], scalar1=0.5,
                                            scalar2=0.5, op0=ALU.mult, op1=ALU.add)
                    tcn = st("tcn")
                    nc.scalar.activation(out=tcn[:], in_=cn, func=AF.Tanh)
                    # h update
                    nc.vector.tensor_tensor(out=h[l][:, 0:12], in0=og[:, 0:12],
                                            in1=tcn[:, 0:12], op=ALU.mult)
                    if l != 1:
                        nc.vector.tensor_tensor(out=h[l][0:96, 12:16],
                                                in0=og[0:96, 12:16],
                                                in1=tcn[0:96, 12:16], op=ALU.mult)
                    # blk3 (units 384:400) products into combo tiles
                    hb3 = st("hb3")
                    nc.gpsimd.tensor_tensor(out=hb3[0:16, 0:4], in0=og[0:16, 12:16],
                                            in1=tcn[0:16, 12:16], op=ALU.mult)
                    if l == 0:
                        nc.gpsimd.tensor_copy(out=cmbA1[96:112, :], in_=hb3[0:16, 0:4])
                        nc.gpsimd.tensor_copy(out=x12[64:80, :], in_=hb3[0:16, 0:4])
                    elif l == 1:
                        nc.gpsimd.tensor_copy(out=phi_aug[32:48, :], in_=hb3[0:16, 0:4])
                        nc.gpsimd.tensor_copy(out=x12[32:48, :], in_=hb3[0:16, 0:4])
                    else:
                        nc.gpsimd.tensor_copy(out=x12[0:16, :], in_=hb3[0:16, 0:4])
                        nc.vector.tensor_copy(
                            out=h3all[:, bass.DynSlice(t * 16, 12)],
                            in_=h[2][:, 0:12])
                        nc.vector.tensor_copy(
                            out=h3all[0:96, bass.DynSlice(t * 16 + 12, 4)],
                            in_=h[2][0:96, 12:16])

                def attention_mm():
                    for b in range(NB):
                        for k in range(4):
                            nc.tensor.matmul(
                                attp[0:1, 30 * b : 30 * b + 30],
                                h[0][:, 4 * k + b : 4 * k + b + 1],
                                sWATT[:, 30 * k : 30 * k + 30],
                                start=(k == 0), stop=(k == 3),
                            )

                def attention():
                    E = _sc["E"]
                    nc.scalar.activation(out=E[:], in_=attp[:], func=AF.Exp)
                    Ev = E[0:1, :].rearrange("p (b x) -> p b x", b=NB)
                    av = attp[0:1, :].rearrange("p (b x) -> p b x", b=NB)
                    kapv = kap[0:1, :].rearrange("p (b k) -> p b k", b=NB)
                    kap2 = _sc["kap2"]
                    k2v = kap2[0:1, :].rearrange("p (b k) -> p b k", b=NB)
                    nc.vector.tensor_tensor(out=k2v, in0=kapv, in1=Ev[:, :, 20:30],
                                            op=ALU.add)
                    nc.gpsimd.tensor_copy(out=kap[:], in_=kap2[:])
                    bk = _sc["bk"]
                    bkv = bk[0:1, :].rearrange("p (b k) -> p b k", b=NB)
                    nc.vector.tensor_tensor(out=bkv, in0=Ev[:, :, 10:20], in1=k2v,
                                            op=ALU.mult)
                    A_ = _sc["A_"]
                    Av = A_[0:1, :].rearrange("p (b k) -> p b k", b=NB)
                    bk2 = _sc["bk2"]
                    bk2v = bk2[0:1, :].rearrange("p (b k) -> p b k", b=NB)
                    nc.vector.tensor_tensor(out=bk2v, in0=bkv, in1=k2v, op=ALU.mult)
                    nc.vector.tensor_tensor(out=Av, in0=av[:, :, 0:10], in1=bk2v,
                                            op=ALU.subtract)
                    B_ = _sc["B_"]
                    nc.vector.tensor_scalar(out=B_[:], in0=bk[:], scalar1=2.0,
                                            scalar2=None, op0=ALU.mult)
                    C_ = _sc["C_"]
                    Cv = C_[0:1, :].rearrange("p (b k) -> p b k", b=NB)
                    nc.vector.tensor_scalar(out=Cv, in0=Ev[:, :, 10:20], scalar1=-1.0,
                                            scalar2=None, op0=ALU.mult)
                    # arg[u,(b,k)] = A + u*B + u^2*C  via 3 accumulating K=1 matmuls
                    nc.tensor.matmul(argp[:], sV3[0:1, 0:UC], A_[:],
                                     start=True, stop=False)
                    nc.tensor.matmul(argp[:], sV3[0:1, UC:2 * UC], B_[:],
                                     start=False, stop=False)
                    nc.tensor.matmul(argp[:], sV3[0:1, 2 * UC:3 * UC], C_[:],
                                     start=False, stop=True)
                    P = _sc["P"]
                    nc.scalar.activation(out=P[:], in_=argp[:], func=AF.Exp)
                    Pv = P[:, :].rearrange("p (b k) -> p b k", b=NB)
                    with nc.allow_low_precision(reason="phi: sum of 10 pos bf16"):
                        nc.vector.tensor_reduce(out=phi_aug[64:114, :], in_=Pv,
                                                axis=mybir.AxisListType.X, op=ALU.add)
                    # wpA = OH_aug^T @ phi_aug : [w | x | h2b3] assembled in psum
                    for b in range(NB):
                        nc.tensor.matmul(
                            wpA[:, b : b + 1],
                            sOHB[:, OHCOLS * b : OHCOLS * b + OHCOLS],
                            phi_aug[0:OHROWS, b : b + 1],
                            start=True, stop=True,
                        )
                    nc.vector.tensor_copy(out=cmbA1[0:92, :], in_=wpA[0:92, :])
                    nc.vector.tensor_copy(out=cmbA0[0:73, :], in_=wpA[0:73, :])

                # seed the activation table (tanh+exp share table 0) on the
                # loop preheader path so the hoist pass can prove it loaded
                warm = _sc["tg"]
                nc.vector.memset(warm[:], 0.0)
                nc.scalar.activation(out=warm[:], in_=warm[:], func=AF.Tanh)
                nc.scalar.activation(out=warm[:], in_=warm[:], func=AF.Exp)

                import os as _os
                import contextlib as _cl
                _pyloop = bool(_os.environ.get("KPYLOOP"))

                def _loop():
                    if _pyloop:
                        return _cl.nullcontext(range(T))
                    return tc.For_i(0, T)

                with _loop() as _ts:
                    _titer = _ts if _pyloop else [_ts]
                    for t in _titer:
                        nc.gpsimd.tensor_copy(out=cmbA0[96:99, :],
                                              in_=sXT[0:3, bass.ts(t, NB)])
                        nc.gpsimd.tensor_copy(out=phi_aug[0:3, :],
                                              in_=sXT[0:3, bass.ts(t, NB)])
                        layer_mms(0, sW1, [(h[0][:, 0:4], 0), (h[0][:, 4:8], 1),
                                           (h[0][:, 8:12], 2), (cmbA0[:], 3),
                                           (x12[:], 4)], first=True, last=True)
                        # recurrence-independent partials of z2/z3 issue now so
                        # the PE stays busy under the cell-0 vector chain
                        layer_mms(1, sW2, [(h[1][:, 0:4], 0), (h[1][:, 4:8], 1),
                                           (h[1][:, 8:12], 2)],
                                  first=True, last=False)
                        layer_mms(2, sW3, [(h[2][:, 0:4], 0), (h[2][:, 4:8], 1),
                                           (h[2][:, 8:12], 2)],
                                  first=True, last=False)
                        cell(0, t)
                        attention_mm()
                        # z2's h1-part right after the cheap att matmuls
                        # (overlaps the attention vector chain on the PE)
                        layer_mms(1, sW2, [(h[0][:, 0:4], 3), (h[0][:, 4:8], 4),
                                           (h[0][:, 8:12], 5)],
                                  first=False, last=False)
                        attention()
                        # z3's w/x part is ready as soon as cmbA1 lands;
                        # issue before cell1 so it overlaps the cell chain
                        layer_mms(2, sW3, [(cmbA1[:], 6)], first=False, last=False)
                        layer_mms(1, sW2, [(cmbA1[:], 6)], first=False, last=True)
                        cell(1, t)
                        layer_mms(2, sW3, [(h[1][:, 0:4], 3), (h[1][:, 4:8], 4),
                                           (h[1][:, 8:12], 5), (x12[:], 7)],
                                  first=False, last=True)
                        cell(2, t)

            # ---- MDN head ----
            # Y1 rows: mu @0:40, eos @64, rho @96:116 ; Y2 rows: pi @0:20, s @32:72
            with tc.tile_pool(name="mpsum", bufs=2, space="PSUM") as mpsum, \
                 tc.tile_pool(name="mscr", bufs=2) as mscr, \
                 tc.tile_pool(name="mones", bufs=1) as mones:
                ones20 = mones.tile([NMIX, 1], F32)
                nc.vector.memset(ones20[:], 1.0)
                ones1_20 = mones.tile([1, NMIX], F32)
                nc.vector.memset(ones1_20[:], 1.0)
                h3v = h3all[:, :].rearrange("p (t x) -> p t x", t=T)
                CC = min(400, T * NB)
                TC = CC // NB
                for ch in range((T + TC - 1) // TC):
                    t0 = TC * ch
                    tn = min(TC, T - t0)
                    cc = tn * NB
                    yp1 = mpsum.tile([128, CC], F32, tag="yp1")
                    yp2 = mpsum.tile([72, CC], F32, tag="yp2")
                    for k in range(4):
                        nc.tensor.matmul(
                            yp1[0:128, 0:cc],
                            sWMDN[:, 200 * k : 200 * k + 128],
                            h3v[:, t0 : t0 + tn, 4 * k : 4 * k + 4],
                            start=(k == 0), stop=(k == 3))
                    for k in range(4):
                        nc.tensor.matmul(
                            yp2[0:72, 0:cc],
                            sWMDN[:, 200 * k + 128 : 200 * k + 200],
                            h3v[:, t0 : t0 + tn, 4 * k : 4 * k + 4],
                            start=(k == 0), stop=(k == 3))
                    o1 = OUTS1[:, NB * t0 : NB * t0 + cc]
                    o2 = OUTS2[:, NB * t0 : NB * t0 + cc]
                    # pi softmax (pi lives at yp2[0:20])
                    epi = mscr.tile([NMIX, CC], F32, tag="epi")
                    nc.scalar.activation(out=epi[0:NMIX, 0:cc], in_=yp2[0:NMIX, 0:cc],
                                         func=AF.Exp)
                    sp = mpsum.tile([1, CC], F32, tag="sp")
                    nc.tensor.matmul(sp[0:1, 0:cc], ones20[:], epi[0:NMIX, 0:cc],
                                     start=True, stop=True)
                    rec = mscr.tile([1, CC], F32, tag="rec")
                    nc.vector.reciprocal(out=rec[0:1, 0:cc], in_=sp[0:1, 0:cc])
                    bp = mpsum.tile([NMIX, CC], F32, tag="bp")
                    nc.tensor.matmul(bp[0:NMIX, 0:cc], ones1_20[:], rec[0:1, 0:cc],
                                     start=True, stop=True)
                    nc.vector.tensor_tensor(out=o2[0:20, :], in0=epi[0:NMIX, 0:cc],
                                            in1=bp[0:NMIX, 0:cc], op=ALU.mult)
                    # mu copy (yp1[0:40])
                    nc.vector.tensor_copy(out=o1[0:40, :], in_=yp1[0:40, 0:cc])
                    # rho tanh (yp1[96:116])
                    nc.scalar.activation(out=o1[96:116, :], in_=yp1[96:116, 0:cc],
                                         func=AF.Tanh)
                    # eos sigmoid via tanh (yp1[64:65])
                    teos = mscr.tile([65, CC], F32, tag="teos")
                    nc.scalar.activation(out=teos[64:65, 0:cc], in_=yp1[64:65, 0:cc],
                                         func=AF.Tanh, scale=0.5)
                    nc.vector.tensor_scalar(out=o1[64:65, :], in0=teos[64:65, 0:cc],
                                            scalar1=0.5, scalar2=0.5,
                                            op0=ALU.mult, op1=ALU.add)
                    # s exp (yp2[32:72], split at quadrant boundary)
                    nc.scalar.activation(out=o2[32:64, :], in_=yp2[32:64, 0:cc],
                                         func=AF.Exp)
                    nc.scalar.activation(out=o2[64:72, :], in_=yp2[64:72, 0:cc],
                                         func=AF.Exp)
            nc.gpsimd.dma_start(out=dOUT1[:], in_=OUTS1[:])
            nc.gpsimd.dma_start(out=dOUT2[:], in_=OUTS2[:])

    nc.compile()
    return nc


def _prep_core(inputs, bsl, T):
    x = np.asarray(inputs['input_strokes'], np.float32)
    chars = np.asarray(inputs['input_chars'])
    lens = np.asarray(inputs['input_char_lens'])

    Wx0 = np.asarray(inputs['Wx0'], np.float32)
    Wh0 = np.asarray(inputs['Wh0'], np.float32)
    b0 = np.asarray(inputs['b0'], np.float32)
    Wx1 = np.asarray(inputs['Wx1'], np.float32)
    Wh1 = np.asarray(inputs['Wh1'], np.float32)
    b1 = np.asarray(inputs['b1'], np.float32)
    Wx2 = np.asarray(inputs['Wx2'], np.float32)
    Wh2 = np.asarray(inputs['Wh2'], np.float32)
    b2 = np.asarray(inputs['b2'], np.float32)

    def pack(kblocks):
        nkt = len(kblocks)
        out = np.zeros((128, nkt * 1600), np.float32)
        for k, blk in enumerate(kblocks):
            out[:, k * 1600:(k + 1) * 1600] = blk
        return out.astype(NPBF)

    def kb(rowmap):
        # rowmap: list of (row_start, W rows [n, 1600])
        blk = np.zeros((128, 1600), np.float32)
        for r0, rows in rowmap:
            blk[r0:r0 + rows.shape[0]] = rows
        return blk

    # L1: k-tiles h1 b0-2 + cmbA0 + x12
    W1 = pack([
        kb([(0, Wh0[0:128])]), kb([(0, Wh0[128:256])]), kb([(0, Wh0[256:384])]),
        kb([(0, Wx0[3:76]), (80, b0[None, :]), (96, Wx0[0:3])]),      # cmbA0
        kb([(64, Wh0[384:400])]),                                      # x12
    ])
    # L2: h2 b0-2 + h1 b0-2 + cmbA1
    W2 = pack([
        kb([(0, Wh1[0:128])]), kb([(0, Wh1[128:256])]), kb([(0, Wh1[256:384])]),
        kb([(0, Wx1[76:204])]), kb([(0, Wx1[204:332])]), kb([(0, Wx1[332:460])]),
        kb([(0, Wx1[3:76]), (73, Wx1[0:3]), (76, Wh1[384:400]),
            (92, b1[None, :]), (96, Wx1[460:476])]),                   # cmbA1
    ])
    # L3: h3 b0-2 + h2 b0-2 + cmbA1(w,x only) + x12
    W3 = pack([
        kb([(0, Wh2[0:128])]), kb([(0, Wh2[128:256])]), kb([(0, Wh2[256:384])]),
        kb([(0, Wx2[76:204])]), kb([(0, Wx2[204:332])]), kb([(0, Wx2[332:460])]),
        kb([(0, Wx2[3:76]), (73, Wx2[0:3])]),                          # cmbA1
        kb([(0, Wh2[384:400]), (32, Wx2[460:476]), (96, b2[None, :])]),  # x12
    ])

    PB = np.zeros((128, 9 * 16), np.float32)
    for l in range(3):
        p = np.asarray(inputs['p%d' % l], np.float32)
        for j in range(3):
            pbv = np.zeros((128, 16), np.float32)
            for blk in range(4):
                n = min(128, 400 - 128 * blk)
                pbv[0:n, 4 * blk : 4 * blk + 4] = p[j][128 * blk : 128 * blk + n, None]
            PB[:, (3 * l + j) * 16 : (3 * l + j) * 16 + 16] = pbv

    XT = np.zeros((3, T * NB), np.float32)
    xs = x[bsl]
    for b in range(NB):
        XT[:, b::NB] = xs[b].T
    WATT = np.zeros((128, 4 * 30), np.float32)
    wa = np.asarray(inputs['W_att'], np.float32)
    for k in range(4):
        n = min(128, 400 - 128 * k)
        WATT[0:n, 30 * k : 30 * k + 30] = wa[128 * k : 128 * k + n]
    WATT[96, 90:120] = np.asarray(inputs['b_att'], np.float32)
    V3 = np.concatenate([np.ones(UC), np.arange(UC),
                         np.arange(UC) ** 2]).astype(np.float32)[None, :]
    # OH_aug: rows 0:3 I3 -> cols 73:76 (x), rows 32:48 I16 -> cols 76:92 (h2b3),
    #         rows 64:114 onehot -> cols 0:73 (w)
    OHB = np.zeros((OHROWS, NB * OHCOLS), np.float32)
    for b, gb in enumerate(bsl):
        oh = np.zeros((OHROWS, OHCOLS), np.float32)
        oh[0:3, 73:76] = np.eye(3)
        oh[32:48, 76:92] = np.eye(16)
        ohw = np.zeros((UC, NCHARS), np.float32)
        ohw[np.arange(UC), chars[gb].astype(int)] = 1.0
        ohw[int(lens[gb]):] = 0.0
        oh[64:114, 0:73] = ohw
        OHB[:, OHCOLS * b : OHCOLS * b + OHCOLS] = oh
    # WMDN: per k-tile block [m1(128) | m2(72)]
    wm = np.asarray(inputs['W_mdn'], np.float32)
    bm = np.asarray(inputs['b_mdn'], np.float32)
    wmf = np.zeros((512, 121), np.float32)
    wmf[0:400] = wm
    wmf[3 * 128 + 96] = bm                  # bias via h3all p96 blk3 == 1.0
    m1 = np.zeros((512, 128), np.float32)
    m2 = np.zeros((512, 72), np.float32)
    m1[:, 0:40] = wmf[:, 20:60]             # mu1, mu2
    m1[:, 64:65] = wmf[:, 120:121]          # eos
    m1[:, 96:116] = wmf[:, 100:120]         # rho
    m2[:, 0:20] = wmf[:, 0:20]              # pi
    m2[:, 32:72] = wmf[:, 60:100]           # s1, s2
    WMDN = np.zeros((128, 4 * 200), np.float32)
    for k in range(4):
        WMDN[:, 200 * k : 200 * k + 128] = m1[128 * k : 128 * k + 128]
        WMDN[:, 200 * k + 128 : 200 * k + 200] = m2[128 * k : 128 * k + 128]
    HB = np.zeros((128, 16), np.float32)
    HB[96, 12:16] = 1.0
    CI = np.zeros((128, 12), np.float32)
    CI[80, 0:4] = 1.0    # cmbA0 bias-one row
    CI[92, 4:8] = 1.0    # cmbA1 bias-one row
    CI[96, 8:12] = 1.0   # x12 bias-one row
    return {'W1': W1, 'W2': W2, 'W3': W3, 'PB': PB,
            'XT': XT.astype(NPBF), 'WATT': WATT.astype(NPBF),
            'V3': V3, 'OHB': OHB.astype(NPBF), 'WMDN': WMDN.astype(NPBF),
            'HB': HB.astype(NPBF), 'CI': CI.astype(NPBF)}


def kernel(**inputs):
    x = np.asarray(inputs['input_strokes'])
    B, T, _ = x.shape
    if T not in _CACHE:
        _CACHE[T] = _build_program(T)
    nc = _CACHE[T]
    in_maps = [_prep_core(inputs, list(range(cr * NB, cr * NB + NB)), T)
               for cr in range(NCORES)]
    res = run_bass_kernel_spmd(nc, in_maps, list(range(NCORES)))
    outs = []
    for cr in range(NCORES):
        O1 = res.results[cr]['OUT1'].reshape(128, T, NB)
        O2 = res.results[cr]['OUT2'].reshape(72, T, NB)
        y = np.empty((NB, T, NOUT), np.float32)
        y[..., 0:20] = O2[0:20].transpose(2, 1, 0)
        y[..., 20:60] = O1[0:40].transpose(2, 1, 0)
        y[..., 60:100] = O2[32:72].transpose(2, 1, 0)
        y[..., 100:120] = O1[96:116].transpose(2, 1, 0)
        y[..., 120:121] = O1[64:65].transpose(2, 1, 0)
        outs.append(y)
    return np.concatenate(outs, 0).astype(np.float32)


# revision 23
# speedup vs baseline: 7.8994x; 1.1398x over previous
import sys
sys.path.insert(0, '/opt/trn_rl_repo')
import numpy as np
import ml_dtypes
import concourse.bass as bass
import concourse.bacc as bacc
import concourse.tile as tile
from concourse import mybir
from concourse.bass_utils import run_bass_kernel_spmd

F32 = mybir.dt.float32
BF16 = mybir.dt.float16
NPBF = np.float16
AF = mybir.ActivationFunctionType
ALU = mybir.AluOpType

U = 400        # LSTM units
KATT = 10     # attention gaussians
NCHARS = 73   # alphabet
NMIX = 20     # GMM components
UC = 50       # char positions
NB = 4        # batch per core
NCORES = 8
NOUT = 6 * NMIX + 1  # 121

# m-tiles: per gate [128,128,128,16] -> 16 m-tiles, psum z [128, 64]
MW = [128, 128, 128, 16] * 4
SOFF = [400 * (m // 4) + 128 * (m % 4) for m in range(16)]   # weight-col offsets

# Moving k-tile layouts (partition-write-alignment legal):
#  cmbA0 [128,4]: w(t-1)@0:73 | b0-one@80 | x(t)@96:99
#  cmbA1 [128,4]: w(t)@0:73 | x(t)@73:76 | h2b3(t-1)@76:92 | b1-one@92 | h1b3(t)@96:112
#  x12   [128,4]: h3b3(t-1)@0:16 | h2b3(t)@32:48 | h1b3(t-1)@64:80 | b2-one@96
#  phi_aug [128,4]: x(t)@0:3 | h2b3(t-1)@32:48 | phi@64:114   (window matmul moving)
#  wpA psum [92,4] = OH_aug^T @ phi_aug = w@0:73 | x@73:76 | h2b3@76:92
OHROWS = 114
OHCOLS = 92

_CACHE = {}


def _build_program(T):
    nc = bacc.Bacc("TRN2", target_bir_lowering=False, debug=False, num_devices=NCORES)

    dW1 = nc.dram_tensor("W1", [128, 5 * 1600], BF16, kind="ExternalInput").ap()
    dW2 = nc.dram_tensor("W2", [128, 7 * 1600], BF16, kind="ExternalInput").ap()
    dW3 = nc.dram_tensor("W3", [128, 8 * 1600], BF16, kind="ExternalInput").ap()
    dPB = nc.dram_tensor("PB", [128, 9 * 16], F32, kind="ExternalInput").ap()
    dXT = nc.dram_tensor("XT", [3, (T + 1) * NB], BF16, kind="ExternalInput").ap()
    dWATT = nc.dram_tensor("WATT", [128, 4 * 30], BF16, kind="ExternalInput").ap()
    dV3 = nc.dram_tensor("V3", [1, 3 * UC], F32, kind="ExternalInput").ap()
    dOHB = nc.dram_tensor("OHB", [OHROWS, NB * OHCOLS], BF16, kind="ExternalInput").ap()
    dWMDN = nc.dram_tensor("WMDN", [128, 4 * 200], BF16, kind="ExternalInput").ap()
    dHB = nc.dram_tensor("HB", [128, 16], BF16, kind="ExternalInput").ap()
    dCI = nc.dram_tensor("CI", [128, 12], BF16, kind="ExternalInput").ap()
    dOUT1 = nc.dram_tensor("OUT1", [128, T * NB], F32, kind="ExternalOutput").ap()
    dOUT2 = nc.dram_tensor("OUT2", [72, T * NB], F32, kind="ExternalOutput").ap()

    with tile.TileContext(nc) as tc:
        with tc.tile_pool(name="statics", bufs=1) as statics, \
             tc.tile_pool(name="states", bufs=1) as states:

            sW1 = statics.tile([128, 5 * 1600], BF16)
            sW2 = statics.tile([128, 7 * 1600], BF16)
            sW3 = statics.tile([128, 8 * 1600], BF16)
            sPB = statics.tile([128, 9 * 16], F32)
            sXT = statics.tile([3, (T + 1) * NB], BF16)
            sWATT = statics.tile([128, 4 * 30], BF16)
            sV3 = statics.tile([1, 3 * UC], F32)
            sOHB = statics.tile([OHROWS, NB * OHCOLS], BF16)
            sWMDN = statics.tile([128, 4 * 200], BF16)
            sHB = statics.tile([128, 16], BF16)
            for dst, src in [(sW1, dW1), (sW2, dW2), (sW3, dW3), (sPB, dPB),
                             (sXT, dXT), (sWATT, dWATT), (sV3, dV3),
                             (sOHB, dOHB), (sWMDN, dWMDN), (sHB, dHB)]:
                nc.gpsimd.dma_start(out=dst[:], in_=src[:])

            h3all = states.tile([128, T * 16], BF16)
            OUTS1 = states.tile([128, T * NB], F32)
            OUTS2 = states.tile([72, T * NB], F32)
            h = [states.tile([128, 16], BF16, name=f"h{i}") for i in range(3)]
            # c stored duplicated [c|c] so the i,f peephole ops fuse to [128,32]
            c = [states.tile([128, 32], F32, name=f"c{i}") for i in range(3)]
            cmbA0 = states.tile([128, NB], BF16, name="cmbA0")
            cmbA1 = states.tile([128, NB], BF16, name="cmbA1")
            x12 = states.tile([128, NB], BF16, name="x12")
            phi_aug = states.tile([128, NB], BF16, name="phi_aug")
            # DMA-init combo tiles: zeros + bias-one rows (avoids unaligned writes)
            nc.gpsimd.dma_start(out=cmbA0[:], in_=dCI[:, 0:4])
            nc.gpsimd.dma_start(out=cmbA1[:], in_=dCI[:, 4:8])
            nc.gpsimd.dma_start(out=x12[:], in_=dCI[:, 8:12])
            kap = states.tile([1, 4 * KATT], F32)  # (b, k)
            for tl in c:
                nc.vector.memset(tl[:], 0.0)
            nc.vector.memset(phi_aug[:], 0.0)
            nc.vector.memset(kap[:], 0.0)
            nc.vector.memset(h3all[:], 0.0)
            # h init: zeros except bias constant 1.0 at (p96, blk3 cols)
            for tl in h:
                nc.vector.tensor_copy(out=tl[:], in_=sHB[:])
            nc.vector.memset(h3all[96:128, :], 1.0)

            with tc.tile_pool(name="psum", bufs=1, space="PSUM") as psum, \
                 tc.tile_pool(name="scratch", bufs=2) as scratch:

                z = [psum.tile([128, 64], F32, name=f"z{i}") for i in range(3)]
                attp = psum.tile([1, NB * 30], F32)
                argp = psum.tile([UC, NB * KATT], F32)
                wpA = psum.tile([OHCOLS, NB], F32)
                for tl in z:
                    nc.vector.memset(tl[:], 0.0)

                _sc = {}
                for _t in ["tg", "m1", "m2", "pc2", "zo_", "to", "og", "tcn",
                           "hb3"]:
                    _sc[_t] = scratch.tile([128, 16], F32, tag=_t, name=_t)
                for _t in ["pcif", "zif_", "tif", "igfg"]:
                    _sc[_t] = scratch.tile([128, 32], F32, tag=_t, name=_t)
                for _t, _shp, _dt in [("E", [1, NB * 30], F32),
                                      ("kap2", [1, NB * KATT], F32),
                                      ("bk", [1, NB * KATT], F32),
                                      ("A_", [1, NB * KATT], F32),
                                      ("bk2", [1, NB * KATT], F32),
                                      ("B_", [1, NB * KATT], F32),
                                      ("C_", [1, NB * KATT], F32),
                                      ("P", [UC, NB * KATT], BF16)]:
                    _sc[_t] = scratch.tile(_shp, _dt, tag=_t, name=_t)

                def layer_mms(l, sW, movs, first, last):
                    # start=True clears has_written for the WHOLE psum bank,
                    # so a step may carry exactly ONE start per z tile: the
                    # very first matmul (bank clear -> every region's first
                    # touch overwrites, later ones accumulate). This lets the
                    # recurrence-independent partials issue early without
                    # corrupting open accumulations in the same bank.
                    for m in range(16):
                        w0, mw = int(SOFF[m]), MW[m]
                        for ki, (mov, kidx) in enumerate(movs):
                            kp = mov.shape[0]
                            nc.tensor.matmul(
                                z[l][0:mw, 4 * m : 4 * m + 4],
                                sW[0:kp, kidx * 1600 + w0 : kidx * 1600 + w0 + mw],
                                mov,
                                start=(first and m == 0 and ki == 0),
                                stop=(last and ki == len(movs) - 1),
                                skip_group_check=True,
                            )

                def st(tag):
                    return _sc[tag]

                def cell(l, t):
                    zt = z[l]
                    zif = zt[:, 0:32]
                    zg, zo = zt[:, 32:48], zt[:, 48:64]
                    p01 = sPB[:, 48 * l : 48 * l + 32]
                    p2 = sPB[:, 48 * l + 32 : 48 * l + 48]
                    ct = c[l]        # [128, 32] duplicated [c|c]
                    cn = ct[:, 0:16]
                    pcif = st("pcif")
                    nc.gpsimd.tensor_tensor(out=pcif[:], in0=p01, in1=ct[:],
                                            op=ALU.mult)
                    zif_ = st("zif_")
                    nc.vector.tensor_tensor(out=zif_[:], in0=zif, in1=pcif[:],
                                            op=ALU.add)
                    tif = st("tif")
                    nc.scalar.activation(out=tif[:], in_=zif_[:], func=AF.Tanh,
                                         scale=0.5)
                    igfg = st("igfg")
                    nc.vector.tensor_scalar(out=igfg[:], in0=tif[:], scalar1=0.5,
                                            scalar2=0.5, op0=ALU.mult, op1=ALU.add)
                    tg = st("tg")
                    nc.scalar.activation(out=tg[:], in_=zg, func=AF.Tanh)
                    m1 = st("m1")
                    nc.vector.tensor_tensor(out=m1[:], in0=igfg[:, 0:16], in1=tg[:],
                                            op=ALU.mult)
                    m2 = st("m2")
                    nc.gpsimd.tensor_tensor(out=m2[:], in0=igfg[:, 16:32], in1=cn,
                                            op=ALU.mult)
                    nc.vector.tensor_tensor(out=cn, in0=m1[:], in1=m2[:], op=ALU.add)
                    # duplicate halves for next step's fused peephole (off-chain)
                    nc.gpsimd.tensor_copy(out=ct[:, 16:32], in_=cn)
                    pc2 = st("pc2")
                    nc.vector.tensor_tensor(out=pc2[:], in0=p2, in1=cn, op=ALU.mult)
                    zo_ = st("zo_")
                    nc.vector.tensor_tensor(out=zo_[:], in0=zo, in1=pc2[:], op=ALU.add)
                    to = st("to")
                    nc.scalar.activation(out=to[:], in_=zo_[:], func=AF.Tanh, scale=0.5)
                    og = st("og")
                    nc.vector.tensor_scalar(out=og[:], in0=to[:], scalar1=0.5,
                                            scalar2=0.5, op0=ALU.mult, op1=ALU.add)
                    tcn = st("tcn")
                    nc.scalar.activation(out=tcn[:], in_=cn, func=AF.Tanh)
                    # h update
                    nc.vector.tensor_tensor(out=h[l][:, 0:12], in0=og[:, 0:12],
                                            in1=tcn[:, 0:12], op=ALU.mult)
                    if l != 1:
                        nc.vector.tensor_tensor(out=h[l][0:96, 12:16],
                                                in0=og[0:96, 12:16],
                                                in1=tcn[0:96, 12:16], op=ALU.mult)
                    # blk3 (units 384:400) products into combo tiles
                    hb3 = st("hb3")
                    nc.gpsimd.tensor_tensor(out=hb3[0:16, 0:4], in0=og[0:16, 12:16],
                                            in1=tcn[0:16, 12:16], op=ALU.mult)
                    if l == 0:
                        nc.gpsimd.tensor_copy(out=cmbA1[96:112, :], in_=hb3[0:16, 0:4])
                        nc.gpsimd.tensor_copy(out=x12[64:80, :], in_=hb3[0:16, 0:4])
                    elif l == 1:
                        nc.gpsimd.tensor_copy(out=phi_aug[32:48, :], in_=hb3[0:16, 0:4])
                        nc.gpsimd.tensor_copy(out=x12[32:48, :], in_=hb3[0:16, 0:4])
                    else:
                        nc.gpsimd.tensor_copy(out=x12[0:16, :], in_=hb3[0:16, 0:4])
                        nc.vector.tensor_copy(
                            out=h3all[:, bass.DynSlice(t * 16, 12)],
                            in_=h[2][:, 0:12])
                        nc.vector.tensor_copy(
                            out=h3all[0:96, bass.DynSlice(t * 16 + 12, 4)],
                            in_=h[2][0:96, 12:16])

                def attention_mm():
                    for b in range(NB):
                        for k in range(4):
                            nc.tensor.matmul(
                                attp[0:1, 30 * b : 30 * b + 30],
                                h[0][:, 4 * k + b : 4 * k + b + 1],
                                sWATT[:, 30 * k : 30 * k + 30],
                                start=(k == 0), stop=(k == 3),
                            )

                def attention():
                    E = _sc["E"]
                    nc.scalar.activation(out=E[:], in_=attp[:], func=AF.Exp)
                    Ev = E[0:1, :].rearrange("p (b x) -> p b x", b=NB)
                    av = attp[0:1, :].rearrange("p (b x) -> p b x", b=NB)
                    kapv = kap[0:1, :].rearrange("p (b k) -> p b k", b=NB)
                    kap2 = _sc["kap2"]
                    k2v = kap2[0:1, :].rearrange("p (b k) -> p b k", b=NB)
                    nc.vector.tensor_tensor(out=k2v, in0=kapv, in1=Ev[:, :, 20:30],
                                            op=ALU.add)
                    nc.gpsimd.tensor_copy(out=kap[:], in_=kap2[:])
                    bk = _sc["bk"]
                    bkv = bk[0:1, :].rearrange("p (b k) -> p b k", b=NB)
                    nc.vector.tensor_tensor(out=bkv, in0=Ev[:, :, 10:20], in1=k2v,
                                            op=ALU.mult)
                    A_ = _sc["A_"]
                    Av = A_[0:1, :].rearrange("p (b k) -> p b k", b=NB)
                    bk2 = _sc["bk2"]
                    bk2v = bk2[0:1, :].rearrange("p (b k) -> p b k", b=NB)
                    nc.vector.tensor_tensor(out=bk2v, in0=bkv, in1=k2v, op=ALU.mult)
                    nc.vector.tensor_tensor(out=Av, in0=av[:, :, 0:10], in1=bk2v,
                                            op=ALU.subtract)
                    B_ = _sc["B_"]
                    nc.vector.tensor_scalar(out=B_[:], in0=bk[:], scalar1=2.0,
                                            scalar2=None, op0=ALU.mult)
                    C_ = _sc["C_"]
                    Cv = C_[0:1, :].rearrange("p (b k) -> p b k", b=NB)
                    nc.vector.tensor_scalar(out=Cv, in0=Ev[:, :, 10:20], scalar1=-1.0,
                                            scalar2=None, op0=ALU.mult)
                    # arg[u,(b,k)] = A + u*B + u^2*C  via 3 accumulating K=1 matmuls
                    nc.tensor.matmul(argp[:], sV3[0:1, 0:UC], A_[:],
                                     start=True, stop=False)
                    nc.tensor.matmul(argp[:], sV3[0:1, UC:2 * UC], B_[:],
                                     start=False, stop=False)
                    nc.tensor.matmul(argp[:], sV3[0:1, 2 * UC:3 * UC], C_[:],
                                     start=False, stop=True)
                    P = _sc["P"]
                    nc.scalar.activation(out=P[:], in_=argp[:], func=AF.Exp)
                    Pv = P[:, :].rearrange("p (b k) -> p b k", b=NB)
                    with nc.allow_low_precision(reason="phi: sum of 10 pos bf16"):
                        nc.vector.tensor_reduce(out=phi_aug[64:114, :], in_=Pv,
                                                axis=mybir.AxisListType.X, op=ALU.add)
                    # wpA = OH_aug^T @ phi_aug : [w | x | h2b3] assembled in psum
                    for b in range(NB):
                        nc.tensor.matmul(
                            wpA[:, b : b + 1],
                            sOHB[:, OHCOLS * b : OHCOLS * b + OHCOLS],
                            phi_aug[0:OHROWS, b : b + 1],
                            start=True, stop=True,
                        )
                    nc.vector.tensor_copy(out=cmbA1[0:92, :], in_=wpA[0:92, :])
                    nc.vector.tensor_copy(out=cmbA0[0:73, :], in_=wpA[0:73, :])

                # seed the activation table (tanh+exp share table 0) on the
                # loop preheader path so the hoist pass can prove it loaded
                warm = _sc["tg"]
                nc.vector.memset(warm[:], 0.0)
                nc.scalar.activation(out=warm[:], in_=warm[:], func=AF.Tanh)
                nc.scalar.activation(out=warm[:], in_=warm[:], func=AF.Exp)

                import os as _os
                import contextlib as _cl
                _pyloop = bool(_os.environ.get("KPYLOOP"))

                def _loop():
                    if _pyloop:
                        return _cl.nullcontext(range(T))
                    return tc.For_i(0, T)

                def z1_mms():
                    layer_mms(0, sW1, [(h[0][:, 0:4], 0), (h[0][:, 4:8], 1),
                                       (h[0][:, 8:12], 2), (cmbA0[:], 3),
                                       (x12[:], 4)], first=True, last=True)

                # prologue: x(0) into the combos, then z1(0). The loop body
                # is rotated one step on z1 (body t emits z1(t+1) mid-body so
                # the PE has work under the cell-1/cell-2 vector chains).
                nc.gpsimd.tensor_copy(out=cmbA0[96:99, :], in_=sXT[0:3, 0:NB])
                nc.gpsimd.tensor_copy(out=phi_aug[0:3, :], in_=sXT[0:3, 0:NB])
                z1_mms()

                with _loop() as _ts:
                    _titer = _ts if _pyloop else [_ts]
                    for t in _titer:
                        # recurrence-independent partials of z2/z3 issue now so
                        # the PE stays busy under the cell-0 vector chain
                        layer_mms(1, sW2, [(h[1][:, 0:4], 0), (h[1][:, 4:8], 1),
                                           (h[1][:, 8:12], 2)],
                                  first=True, last=False)
                        layer_mms(2, sW3, [(h[2][:, 0:4], 0), (h[2][:, 4:8], 1),
                                           (h[2][:, 8:12], 2)],
                                  first=True, last=False)
                        cell(0, t)
                        attention_mm()
                        # z2's h1-part right after the cheap att matmuls
                        # (overlaps the attention vector chain on the PE)
                        layer_mms(1, sW2, [(h[0][:, 0:4], 3), (h[0][:, 4:8], 4),
                                           (h[0][:, 8:12], 5)],
                                  first=False, last=False)
                        attention()
                        # stage x(t+1) after the window matmuls consumed x(t)
                        nc.gpsimd.tensor_copy(out=cmbA0[96:99, :],
                                              in_=sXT[0:3, bass.ts(t + 1, NB)])
                        nc.gpsimd.tensor_copy(out=phi_aug[0:3, :],
                                              in_=sXT[0:3, bass.ts(t + 1, NB)])
                        # z3's w/x part is ready as soon as cmbA1 lands;
                        # issue before cell1 so it overlaps the cell chain
                        layer_mms(2, sW3, [(cmbA1[:], 6)], first=False, last=False)
                        layer_mms(1, sW2, [(cmbA1[:], 6)], first=False, last=True)
                        # z1(t+1): h1(t), w(t), x(t+1) all ready; fills the PE
                        # during the cell-1 chain
                        z1_mms()
                        cell(1, t)
                        layer_mms(2, sW3, [(h[1][:, 0:4], 3), (h[1][:, 4:8], 4),
                                           (h[1][:, 8:12], 5), (x12[:], 7)],
                                  first=False, last=True)
                        cell(2, t)

            # ---- MDN head ----
            # Y1 rows: mu @0:40, eos @64, rho @96:116 ; Y2 rows: pi @0:20, s @32:72
            with tc.tile_pool(name="mpsum", bufs=2, space="PSUM") as mpsum, \
                 tc.tile_pool(name="mscr", bufs=2) as mscr, \
                 tc.tile_pool(name="mones", bufs=1) as mones:
                ones20 = mones.tile([NMIX, 1], F32)
                nc.vector.memset(ones20[:], 1.0)
                ones1_20 = mones.tile([1, NMIX], F32)
                nc.vector.memset(ones1_20[:], 1.0)
                h3v = h3all[:, :].rearrange("p (t x) -> p t x", t=T)
                CC = min(400, T * NB)
                TC = CC // NB
                for ch in range((T + TC - 1) // TC):
                    t0 = TC * ch
                    tn = min(TC, T - t0)
                    cc = tn * NB
                    yp1 = mpsum.tile([128, CC], F32, tag="yp1")
                    yp2 = mpsum.tile([72, CC], F32, tag="yp2")
                    for k in range(4):
                        nc.tensor.matmul(
                            yp1[0:128, 0:cc],
                            sWMDN[:, 200 * k : 200 * k + 128],
                            h3v[:, t0 : t0 + tn, 4 * k : 4 * k + 4],
                            start=(k == 0), stop=(k == 3))
                    for k in range(4):
                        nc.tensor.matmul(
                            yp2[0:72, 0:cc],
                            sWMDN[:, 200 * k + 128 : 200 * k + 200],
                            h3v[:, t0 : t0 + tn, 4 * k : 4 * k + 4],
                            start=(k == 0), stop=(k == 3))
                    o1 = OUTS1[:, NB * t0 : NB * t0 + cc]
                    o2 = OUTS2[:, NB * t0 : NB * t0 + cc]
                    # pi softmax (pi lives at yp2[0:20])
                    epi = mscr.tile([NMIX, CC], F32, tag="epi")
                    nc.scalar.activation(out=epi[0:NMIX, 0:cc], in_=yp2[0:NMIX, 0:cc],
                                         func=AF.Exp)
                    sp = mpsum.tile([1, CC], F32, tag="sp")
                    nc.tensor.matmul(sp[0:1, 0:cc], ones20[:], epi[0:NMIX, 0:cc],
                                     start=True, stop=True)
                    rec = mscr.tile([1, CC], F32, tag="rec")
                    nc.vector.reciprocal(out=rec[0:1, 0:cc], in_=sp[0:1, 0:cc])
                    bp = mpsum.tile([NMIX, CC], F32, tag="bp")
                    nc.tensor.matmul(bp[0:NMIX, 0:cc], ones1_20[:], rec[0:1, 0:cc],
                                     start=True, stop=True)
                    nc.vector.tensor_tensor(out=o2[0:20, :], in0=epi[0:NMIX, 0:cc],
                                            in1=bp[0:NMIX, 0:cc], op=ALU.mult)
                    # mu copy (yp1[0:40])
                    nc.vector.tensor_copy(out=o1[0:40, :], in_=yp1[0:40, 0:cc])
                    # rho tanh (yp1[96:116])
                    nc.scalar.activation(out=o1[96:116, :], in_=yp1[96:116, 0:cc],
                                         func=AF.Tanh)
                    # eos sigmoid via tanh (yp1[64:65])
                    teos = mscr.tile([65, CC], F32, tag="teos")
                    nc.scalar.activation(out=teos[64:65, 0:cc], in_=yp1[64:65, 0:cc],
                                         func=AF.Tanh, scale=0.5)
                    nc.vector.tensor_scalar(out=o1[64:65, :], in0=teos[64:65, 0:cc],
                                            scalar1=0.5, scalar2=0.5,
                                            op0=ALU.mult, op1=ALU.add)
                    # s exp (yp2[32:72], split at quadrant boundary)
                    nc.scalar.activation(out=o2[32:64, :], in_=yp2[32:64, 0:cc],
                                         func=AF.Exp)
                    nc.scalar.activation(out=o2[64:72, :], in_=yp2[64:72, 0:cc],
                                         func=AF.Exp)
            nc.gpsimd.dma_start(out=dOUT1[:], in_=OUTS1[:])
            nc.gpsimd.dma_start(out=dOUT2[:], in_=OUTS2[:])

    nc.compile()
    return nc


def _prep_core(inputs, bsl, T):
    x = np.asarray(inputs['input_strokes'], np.float32)
    chars = np.asarray(inputs['input_chars'])
    lens = np.asarray(inputs['input_char_lens'])

    Wx0 = np.asarray(inputs['Wx0'], np.float32)
    Wh0 = np.asarray(inputs['Wh0'], np.float32)
    b0 = np.asarray(inputs['b0'], np.float32)
    Wx1 = np.asarray(inputs['Wx1'], np.float32)
    Wh1 = np.asarray(inputs['Wh1'], np.float32)
    b1 = np.asarray(inputs['b1'], np.float32)
    Wx2 = np.asarray(inputs['Wx2'], np.float32)
    Wh2 = np.asarray(inputs['Wh2'], np.float32)
    b2 = np.asarray(inputs['b2'], np.float32)

    def pack(kblocks):
        nkt = len(kblocks)
        out = np.zeros((128, nkt * 1600), np.float32)
        for k, blk in enumerate(kblocks):
            out[:, k * 1600:(k + 1) * 1600] = blk
        return out.astype(NPBF)

    def kb(rowmap):
        # rowmap: list of (row_start, W rows [n, 1600])
        blk = np.zeros((128, 1600), np.float32)
        for r0, rows in rowmap:
            blk[r0:r0 + rows.shape[0]] = rows
        return blk

    # L1: k-tiles h1 b0-2 + cmbA0 + x12
    W1 = pack([
        kb([(0, Wh0[0:128])]), kb([(0, Wh0[128:256])]), kb([(0, Wh0[256:384])]),
        kb([(0, Wx0[3:76]), (80, b0[None, :]), (96, Wx0[0:3])]),      # cmbA0
        kb([(64, Wh0[384:400])]),                                      # x12
    ])
    # L2: h2 b0-2 + h1 b0-2 + cmbA1
    W2 = pack([
        kb([(0, Wh1[0:128])]), kb([(0, Wh1[128:256])]), kb([(0, Wh1[256:384])]),
        kb([(0, Wx1[76:204])]), kb([(0, Wx1[204:332])]), kb([(0, Wx1[332:460])]),
        kb([(0, Wx1[3:76]), (73, Wx1[0:3]), (76, Wh1[384:400]),
            (92, b1[None, :]), (96, Wx1[460:476])]),                   # cmbA1
    ])
    # L3: h3 b0-2 + h2 b0-2 + cmbA1(w,x only) + x12
    W3 = pack([
        kb([(0, Wh2[0:128])]), kb([(0, Wh2[128:256])]), kb([(0, Wh2[256:384])]),
        kb([(0, Wx2[76:204])]), kb([(0, Wx2[204:332])]), kb([(0, Wx2[332:460])]),
        kb([(0, Wx2[3:76]), (73, Wx2[0:3])]),                          # cmbA1
        kb([(0, Wh2[384:400]), (32, Wx2[460:476]), (96, b2[None, :])]),  # x12
    ])

    PB = np.zeros((128, 9 * 16), np.float32)
    for l in range(3):
        p = np.asarray(inputs['p%d' % l], np.float32)
        for j in range(3):
            pbv = np.zeros((128, 16), np.float32)
            for blk in range(4):
                n = min(128, 400 - 128 * blk)
                pbv[0:n, 4 * blk : 4 * blk + 4] = p[j][128 * blk : 128 * blk + n, None]
            PB[:, (3 * l + j) * 16 : (3 * l + j) * 16 + 16] = pbv

    XT = np.zeros((3, (T + 1) * NB), np.float32)
    xs = x[bsl]
    for b in range(NB):
        XT[:, b:T * NB:NB] = xs[b].T
    WATT = np.zeros((128, 4 * 30), np.float32)
    wa = np.asarray(inputs['W_att'], np.float32)
    for k in range(4):
        n = min(128, 400 - 128 * k)
        WATT[0:n, 30 * k : 30 * k + 30] = wa[128 * k : 128 * k + n]
    WATT[96, 90:120] = np.asarray(inputs['b_att'], np.float32)
    V3 = np.concatenate([np.ones(UC), np.arange(UC),
                         np.arange(UC) ** 2]).astype(np.float32)[None, :]
    # OH_aug: rows 0:3 I3 -> cols 73:76 (x), rows 32:48 I16 -> cols 76:92 (h2b3),
    #         rows 64:114 onehot -> cols 0:73 (w)
    OHB = np.zeros((OHROWS, NB * OHCOLS), np.float32)
    for b, gb in enumerate(bsl):
        oh = np.zeros((OHROWS, OHCOLS), np.float32)
        oh[0:3, 73:76] = np.eye(3)
        oh[32:48, 76:92] = np.eye(16)
        ohw = np.zeros((UC, NCHARS), np.float32)
        ohw[np.arange(UC), chars[gb].astype(int)] = 1.0
        ohw[int(lens[gb]):] = 0.0
        oh[64:114, 0:73] = ohw
        OHB[:, OHCOLS * b : OHCOLS * b + OHCOLS] = oh
    # WMDN: per k-tile block [m1(128) | m2(72)]
    wm = np.asarray(inputs['W_mdn'], np.float32)
    bm = np.asarray(inputs['b_mdn'], np.float32)
    wmf = np.zeros((512, 121), np.float32)
    wmf[0:400] = wm
    wmf[3 * 128 + 96] = bm                  # bias via h3all p96 blk3 == 1.0
    m1 = np.zeros((512, 128), np.float32)
    m2 = np.zeros((512, 72), np.float32)
    m1[:, 0:40] = wmf[:, 20:60]             # mu1, mu2
    m1[:, 64:65] = wmf[:, 120:121]          # eos
    m1[:, 96:116] = wmf[:, 100:120]         # rho
    m2[:, 0:20] = wmf[:, 0:20]              # pi
    m2[:, 32:72] = wmf[:, 60:100]           # s1, s2
    WMDN = np.zeros((128, 4 * 200), np.float32)
    for k in range(4):
        WMDN[:, 200 * k : 200 * k + 128] = m1[128 * k : 128 * k + 128]
        WMDN[:, 200 * k + 128 : 200 * k + 200] = m2[128 * k : 128 * k + 128]
    HB = np.zeros((128, 16), np.float32)
    HB[96, 12:16] = 1.0
    CI = np.zeros((128, 12), np.float32)
    CI[80, 0:4] = 1.0    # cmbA0 bias-one row
    CI[92, 4:8] = 1.0    # cmbA1 bias-one row
    CI[96, 8:12] = 1.0   # x12 bias-one row
    return {'W1': W1, 'W2': W2, 'W3': W3, 'PB': PB,
            'XT': XT.astype(NPBF), 'WATT': WATT.astype(NPBF),
            'V3': V3, 'OHB': OHB.astype(NPBF), 'WMDN': WMDN.astype(NPBF),
            'HB': HB.astype(NPBF), 'CI': CI.astype(NPBF)}


def kernel(**inputs):
    x = np.asarray(inputs['input_strokes'])
    B, T, _ = x.shape
    if T not in _CACHE:
        _CACHE[T] = _build_program(T)
    nc = _CACHE[T]
    in_maps = [_prep_core(inputs, list(range(cr * NB, cr * NB + NB)), T)
               for cr in range(NCORES)]
    res = run_bass_kernel_spmd(nc, in_maps, list(range(NCORES)))
    outs = []
    for cr in range(NCORES):
        O1 = res.results[cr]['OUT1'].reshape(128, T, NB)
        O2 = res.results[cr]['OUT2'].reshape(72, T, NB)
        y = np.empty((NB, T, NOUT), np.float32)
        y[..., 0:20] = O2[0:20].transpose(2, 1, 0)
        y[..., 20:60] = O1[0:40].transpose(2, 1, 0)
        y[..., 60:100] = O2[32:72].transpose(2, 1, 0)
        y[..., 100:120] = O1[96:116].transpose(2, 1, 0)
        y[..., 120:121] = O1[64:65].transpose(2, 1, 0)
        outs.append(y)
    return np.concatenate(outs, 0).astype(np.float32)
